# revision 1
# baseline (speedup 1.0000x reference)
"""Multi-head attention (B=4, N=2048, C=1024, H=16, D=64) on 8 TRN2 cores.

Sharding: core c -> batch b = c%4, head-group g = c//4 (local heads 0..7 are
global heads 8g..8g+7).  Each core computes its head group's contribution to
the output projection for its batch; host sums core b + core b+4 and adds
const_row = qkv_b[2048:] @ proj_w + proj_b (V-bias folds exactly through the
row-normalized attention: attn @ (1*bv^T) = 1*bv^T).

Device layouts (per core):
  xT   [1024, 2048]  x[b].T (C x N), fp32r-prerounded
  wcat [1024, 1536]  [Wq | Wk | Wv] cols for this head group, fp32r
  qb/kb [128, 4]     bias chunk pr in column pr
  pw   [512, 1024]   proj_w rows for this head group's channels, fp32r
  out  [2048, 1024]  partial projection output

Phase 1: QKV projection.  Q_T/K_T [128, 4, 2048]: partition = dim-in-pair
(2 heads x 64), free = token.  V_sb [128, 16, 8, 65]: partition = token-in-
block, per (block t, head h) a [128, 65] lhsT whose col 64 is ones (denom).
Phase 2: per query block (512) x head pair: row-packed K=64 score matmuls
-> PSUM [128 keys, 1024] -> one ACT exp (scale=0.125) -> P fp32r ->
PV matmuls accumulate oaug [65, 512]; row 64 = denominator; DVE reciprocal +
DRAM-bounce broadcast + tensor_tensor multiply -> O_qb [128, 4, 512] fp32r;
projection matmuls contract the 4 pairs -> out.
"""

import sys

sys.path.insert(0, "/opt/trn_rl_repo")

from contextlib import ExitStack

import numpy as np

from concourse import bacc, mybir, tile
from concourse.bass_utils import run_bass_kernel_spmd

F32 = mybir.dt.float32
F32R = mybir.dt.float32r
EXP = mybir.ActivationFunctionType.Exp
ADD = mybir.AluOpType.add
MULT = mybir.AluOpType.mult

B, N, C, H, D = 4, 2048, 1024, 16, 64
SCALE = 0.125


def _round_fp32r(a: np.ndarray) -> np.ndarray:
    b = np.ascontiguousarray(a, dtype=np.float32).view(np.uint32).astype(np.uint64)
    lsb = (b >> np.uint64(12)) & np.uint64(1)
    b = (b + np.uint64(0x7FF) + lsb) & np.uint64(0xFFFFF000)
    return b.astype(np.uint32).view(np.float32)


def _build():
    nc = bacc.Bacc("TRN2", target_bir_lowering=False, debug=False)
    xT = nc.dram_tensor("xT", [1024, 2048], F32, kind="ExternalInput").ap()
    wcat = nc.dram_tensor("wcat", [1024, 1536], F32, kind="ExternalInput").ap()
    qb = nc.dram_tensor("qb", [128, 4], F32, kind="ExternalInput").ap()
    kb = nc.dram_tensor("kb", [128, 4], F32, kind="ExternalInput").ap()
    pw = nc.dram_tensor("pw", [512, 1024], F32, kind="ExternalInput").ap()
    out = nc.dram_tensor("out", [2048, 1024], F32, kind="ExternalOutput").ap()
    scratch = nc.dram_tensor("scratch", [32, 512], F32).ap()

    with tile.TileContext(nc) as tc, ExitStack() as ctx:
        sb = ctx.enter_context(tc.tile_pool(name="sb", bufs=1))
        ps = ctx.enter_context(tc.tile_pool(name="ps", bufs=1, space="PSUM"))

        w_sb = sb.tile([128, 8, 1536], F32R, tag="w")
        Q_T = sb.tile([128, 4, 2048], F32R, tag="qt")
        K_T = sb.tile([128, 4, 2048], F32R, tag="kt")
        V_sb = sb.tile([128, 16, 8, 65], F32R, tag="v")
        qb_sb = sb.tile([128, 4], F32, tag="qb")
        kb_sb = sb.tile([128, 4], F32, tag="kb")
        zc = sb.tile([128, 8, 1], F32, tag="zc")
        onec = sb.tile([128, 1], F32, tag="onec")

        for j in range(8):
            nc.sync.dma_start(w_sb[:, j, :], wcat[j * 128:(j + 1) * 128, :].bitcast(F32R))
        nc.sync.dma_start(qb_sb[:], qb[:])
        nc.sync.dma_start(kb_sb[:], kb[:])
        nc.vector.memset(zc[:], 0.0)
        nc.vector.memset(onec[:], 1.0)
        for t in range(16):
            nc.vector.tensor_scalar(out=V_sb[:, t, :, 64:65], in0=zc[:],
                                    scalar1=onec[:], scalar2=None, op0=ADD)

        # Phase 1: QKV projection, 8 token blocks of 256
        for nb in range(8):
            slab = sb.tile([128, 8, 256], F32R, tag="xslab", bufs=2)
            for j in range(8):
                nc.sync.dma_start(
                    slab[:, j, :],
                    xT[j * 128:(j + 1) * 128, nb * 256:(nb + 1) * 256].bitcast(F32R))
            for pr in range(4):
                acc = ps.tile([128, 512], F32, tag="stage", bufs=2)
                for j in range(8):
                    nc.tensor.matmul(acc[:, 0:256],
                                     w_sb[:, j, pr * 128:(pr + 1) * 128],
                                     slab[:, j, :], start=(j == 0), stop=(j == 7))
                nc.vector.tensor_scalar(out=Q_T[:, pr, nb * 256:(nb + 1) * 256],
                                        in0=acc[:, 0:256],
                                        scalar1=qb_sb[:, pr:pr + 1],
                                        scalar2=None, op0=ADD)
            for pr in range(4):
                acc = ps.tile([128, 512], F32, tag="stage", bufs=2)
                for j in range(8):
                    nc.tensor.matmul(acc[:, 0:256],
                                     w_sb[:, j, 512 + pr * 128:512 + (pr + 1) * 128],
                                     slab[:, j, :], start=(j == 0), stop=(j == 7))
                nc.vector.tensor_scalar(out=K_T[:, pr, nb * 256:(nb + 1) * 256],
                                        in0=acc[:, 0:256],
                                        scalar1=kb_sb[:, pr:pr + 1],
                                        scalar2=None, op0=ADD)
            for kt2 in range(2):
                t = nb * 2 + kt2
                acc = ps.tile([128, 512], F32, tag="stage", bufs=2)
                for j in range(8):
                    nc.tensor.matmul(acc[:],
                                     slab[:, j, kt2 * 128:(kt2 + 1) * 128],
                                     w_sb[:, j, 1024:1536],
                                     start=(j == 0), stop=(j == 7))
                nc.vector.tensor_copy(out=V_sb[:, t, :, 0:64],
                                      in_=acc[:].rearrange("p (h d) -> p h d", h=8))

        # Phase 2: attention + projection
        pw_sb = sb.tile([128, 4, 1024], F32R, tag="w")
        for pr in range(4):
            nc.sync.dma_start(pw_sb[:, pr, :],
                              pw[pr * 128:(pr + 1) * 128, :].bitcast(F32R))

        for qb_i in range(4):
            q0 = qb_i * 512
            O_qb = sb.tile([128, 4, 512], F32R, tag="xslab", bufs=2)
            for pr in range(4):
                oaug0 = ps.tile([65, 512], F32, tag="oaug", bufs=4)
                oaug1 = ps.tile([65, 512], F32, tag="oaug", bufs=4)
                staged = []
                for tg in range(9):
                    if tg < 8:
                        t0, t1 = 2 * tg, 2 * tg + 1
                        stage0 = ps.tile([128, 1024], F32, tag="stage", bufs=2)
                        stage1 = ps.tile([128, 1024], F32, tag="stage", bufs=2)
                        # scores S^T [keys, queries]; heads (2pr,2pr+1) row-packed
                        nc.tensor.matmul(stage0[:, 0:512],
                                         K_T[0:64, pr, t0 * 128:(t0 + 1) * 128],
                                         Q_T[0:64, pr, q0:q0 + 512],
                                         start=True, stop=True, tile_position=(0, 0))
                        nc.tensor.matmul(stage1[:, 0:512],
                                         K_T[64:128, pr, t0 * 128:(t0 + 1) * 128],
                                         Q_T[64:128, pr, q0:q0 + 512],
                                         start=True, stop=True, tile_position=(64, 0))
                        nc.tensor.matmul(stage0[:, 512:1024],
                                         K_T[0:64, pr, t1 * 128:(t1 + 1) * 128],
                                         Q_T[0:64, pr, q0:q0 + 512],
                                         start=True, stop=True, tile_position=(0, 0))
                        nc.tensor.matmul(stage1[:, 512:1024],
                                         K_T[64:128, pr, t1 * 128:(t1 + 1) * 128],
                                         Q_T[64:128, pr, q0:q0 + 512],
                                         start=True, stop=True, tile_position=(64, 0))
                    if tg >= 1:
                        # PV lags S by one tg so exp overlaps the next S pair
                        pP0, pP1, pt0, pt1 = staged[tg - 1]
                        st, sp = (tg - 1 == 0), (tg - 1 == 7)
                        nc.tensor.matmul(oaug0[:], V_sb[:, pt0, 2 * pr, :],
                                         pP0[:, 0:512], start=st, stop=False)
                        nc.tensor.matmul(oaug0[:], V_sb[:, pt1, 2 * pr, :],
                                         pP0[:, 512:1024], start=False, stop=sp)
                        nc.tensor.matmul(oaug1[:], V_sb[:, pt0, 2 * pr + 1, :],
                                         pP1[:, 0:512], start=st, stop=False)
                        nc.tensor.matmul(oaug1[:], V_sb[:, pt1, 2 * pr + 1, :],
                                         pP1[:, 512:1024], start=False, stop=sp)
                    if tg < 8:
                        P0 = sb.tile([128, 1024], F32R, tag="p", bufs=3)
                        P1 = sb.tile([128, 1024], F32R, tag="p", bufs=3)
                        nc.scalar.activation(P0[:], stage0[:], EXP,
                                             bias=0.0, scale=SCALE)
                        nc.scalar.activation(P1[:], stage1[:], EXP,
                                             bias=0.0, scale=SCALE)
                        staged.append((P0, P1, t0, t1))
                for hh, oaug in ((0, oaug0), (1, oaug1)):
                    row = qb_i * 8 + pr * 2 + hh
                    rc = sb.tile([128, 512], F32, tag="recip", bufs=2)
                    nc.vector.reciprocal(rc[64:65, :], oaug[64:65, :])
                    nc.sync.dma_start(scratch[row:row + 1, :], rc[64:65, :])
                    rb = sb.tile([64, 512], F32, tag="rb", bufs=2)
                    nc.sync.dma_start(
                        rb[:], scratch[row:row + 1, :].to_broadcast((64, 512)))
                    nc.vector.tensor_tensor(out=O_qb[hh * 64:(hh + 1) * 64, pr, :],
                                            in0=oaug[0:64, :], in1=rb[:], op=MULT)
            for ns in range(4):
                for co in range(2):
                    pj = ps.tile([128, 512], F32, tag="oaug", bufs=4)
                    for pr in range(4):
                        nc.tensor.matmul(pj[:],
                                         O_qb[:, pr, ns * 128:(ns + 1) * 128],
                                         pw_sb[:, pr, co * 512:(co + 1) * 512],
                                         start=(pr == 0), stop=(pr == 3))
                    so = sb.tile([128, 512], F32, tag="stout", bufs=2)
                    nc.vector.tensor_copy(out=so[:], in_=pj[:])
                    nc.sync.dma_start(
                        out[q0 + ns * 128:q0 + (ns + 1) * 128,
                            co * 512:(co + 1) * 512], so[:])
    return nc


def _prepare_in_maps(x, qkv_w, qkv_b, proj_w):
    xr = _round_fp32r(x)
    wr = _round_fp32r(qkv_w)
    pwr = _round_fp32r(proj_w)
    qkv_b = np.asarray(qkv_b, dtype=np.float32)
    in_maps = []
    for c in range(8):
        b, g = c % 4, c // 4
        w0 = 512 * g
        in_maps.append({
            "xT": np.ascontiguousarray(xr[b].T),
            "wcat": np.ascontiguousarray(np.concatenate(
                [wr[:, w0:w0 + 512],
                 wr[:, 1024 + w0:1024 + w0 + 512],
                 wr[:, 2048 + w0:2048 + w0 + 512]], axis=1)),
            "qb": np.ascontiguousarray(qkv_b[w0:w0 + 512].reshape(4, 128).T),
            "kb": np.ascontiguousarray(
                qkv_b[1024 + w0:1024 + w0 + 512].reshape(4, 128).T),
            "pw": np.ascontiguousarray(pwr[w0:w0 + 512, :]),
        })
    return in_maps


def _gather(parts, qkv_b, proj_w, proj_b):
    const_row = (np.asarray(qkv_b)[2048:].astype(np.float64)
                 @ np.asarray(proj_w).astype(np.float64)
                 + np.asarray(proj_b).astype(np.float64))
    out = np.empty((B, N, C), np.float32)
    for b in range(B):
        out[b] = (parts[b].astype(np.float64) + parts[b + 4].astype(np.float64)
                  + const_row).astype(np.float32)
    return out


def kernel(**inputs: np.ndarray) -> np.ndarray:
    x = np.asarray(inputs["x"], dtype=np.float32)
    qkv_w = np.asarray(inputs["qkv_w"], dtype=np.float32)
    qkv_b = np.asarray(inputs["qkv_b"], dtype=np.float32)
    proj_w = np.asarray(inputs["proj_w"], dtype=np.float32)
    proj_b = np.asarray(inputs["proj_b"], dtype=np.float32)

    in_maps = _prepare_in_maps(x, qkv_w, qkv_b, proj_w)
    nc = _build()
    nc.finalize()
    res = run_bass_kernel_spmd(nc, in_maps, list(range(8)))
    parts = [res.results[c]["out"] for c in range(8)]
    return _gather(parts, qkv_b, proj_w, proj_b)


if __name__ == "__main__":
    import tempfile
    import time

    from concourse.bass_utils import compile_bass_kernel

    t0 = time.time()
    nc = _build()
    nc.compile()
    with tempfile.TemporaryDirectory() as td:
        compile_bass_kernel(nc, td, neff_name="k.neff")
    print(f"COMPILE OK ({time.time() - t0:.0f}s)", flush=True)



# revision 21
# speedup vs baseline: 1.2330x; 1.2330x over previous
"""Multi-head attention (B=4, N=2048, C=1024, H=16, D=64) on 8 TRN2 cores.

Sharding: core c -> batch b = c%4, head-group g = c//4 (local heads 0..7 are
global heads 8g..8g+7).  Each core computes its head group's contribution to
the output projection for its batch; host sums core b + core b+4 and adds
const_row = qkv_b[2048:] @ proj_w + proj_b (V-bias folds exactly through the
row-normalized attention: attn @ (1*bv^T) = 1*bv^T).

v2: all-bf16 datapath (inputs pre-cast on host), software-pipelined schedule:
QKV projection for head-pair pr+1 is interleaved into the attention tg-loop of
head-pair pr so the scalar engine's exp stream (the phase-2 co-bottleneck)
overlaps the tensor engine's QKV matmuls instead of idling during a separate
phase 1.  Projection for query block qb is interleaved into attention of
(pr=3, qb+1).  Softmax normalization moved off the critical path: oaug PSUM is
copied to SBUF immediately (freeing the PSUM bank for the next query block),
then reciprocal_approx_fast + DRAM-bounce broadcast + multiply trail behind.

Per-core device layout:
  x_sb[j]  [128, 2048] bf16   xT rows j*128.. (C x N), j = 0..7
  wq/wk/wv_sb[j] [128, 512]   W columns for this head group, per input-row j
  QT/KT[pr] [128, 2048] bf16  partition = dim-in-pair (2 heads x 64), free = tok
  VT[pr]   [128, 16, 2, 65]   partition = token-in-block; col 64 = ones (denom)
  OT[pr]   [128, 2048] bf16   normalized attention output, chan-pair x token
  out      [2048, 1024] f32   partial projection output
"""

import sys

sys.path.insert(0, "/opt/trn_rl_repo")

from collections import deque
from contextlib import ExitStack

import numpy as np
from ml_dtypes import bfloat16

from concourse import bacc, mybir, tile
from concourse.bass_utils import run_bass_kernel_spmd

F32 = mybir.dt.float32
BF16 = mybir.dt.bfloat16
EXP = mybir.ActivationFunctionType.Exp
ADD = mybir.AluOpType.add
MULT = mybir.AluOpType.mult

B, N, C, H, D = 4, 2048, 1024, 16, 64
SCALE = 0.125


def _build(dbg=False):
    nc = bacc.Bacc("TRN2", target_bir_lowering=False, debug=False)
    xT = nc.dram_tensor("xT", [1024, 2048], BF16, kind="ExternalInput").ap()
    wcat = nc.dram_tensor("wcat", [1024, 1536], BF16, kind="ExternalInput").ap()
    qbias = nc.dram_tensor("qb", [128, 4], F32, kind="ExternalInput").ap()
    kbias = nc.dram_tensor("kb", [128, 4], F32, kind="ExternalInput").ap()
    pw = nc.dram_tensor("pw", [512, 1024], BF16, kind="ExternalInput").ap()
    out = nc.dram_tensor("out", [2048, 1024], F32, kind="ExternalOutput").ap()
    scratch = nc.dram_tensor("scratch", [32, 512], F32).ap()
    if dbg:
        dbgb = nc.dram_tensor("dbgb", [8, 2048], BF16,
                              kind="ExternalOutput").ap()
        dbgf = nc.dram_tensor("dbgf", [66, 512], F32,
                              kind="ExternalOutput").ap()

    with tile.TileContext(nc) as tc, ExitStack() as ctx:
        sb = ctx.enter_context(tc.tile_pool(name="sb", bufs=1))
        ps = ctx.enter_context(tc.tile_pool(name="ps", bufs=1, space="PSUM"))

        x_sb = [sb.tile([128, 2048], BF16, name=f"x{j}") for j in range(8)]
        wq_sb = [sb.tile([128, 512], BF16, name=f"wq{j}") for j in range(8)]
        wk_sb = [sb.tile([128, 512], BF16, name=f"wk{j}") for j in range(8)]
        wv_sb = [sb.tile([128, 512], BF16, name=f"wv{j}") for j in range(8)]
        pw_sb = sb.tile([128, 4, 1024], BF16, tag="pw")
        QT = [sb.tile([128, 2048], BF16, name=f"QT{p}") for p in range(4)]
        KT = [sb.tile([128, 2048], BF16, name=f"KT{p}") for p in range(4)]
        VT = [sb.tile([128, 16, 2, 65], BF16, name=f"VT{p}") for p in range(4)]
        OT = [sb.tile([128, 2048], BF16, name=f"OT{p}") for p in range(4)]
        qb_sb = sb.tile([128, 4], F32, tag="qb")
        kb_sb = sb.tile([128, 4], F32, tag="kb")
        zc = sb.tile([128, 16, 2, 1], F32, tag="zc")
        onec = sb.tile([128, 1], F32, tag="onec")

        # loads ordered by first use: Q weights + x stream, then K, V, proj.
        # x goes out on the scalar engine's DMA queue, weights on sync's, so
        # the two streams transfer in parallel and the first QKV matmul can
        # start ~8us earlier.
        for j in range(8):
            nc.sync.dma_start(wq_sb[j][:], wcat[j * 128:(j + 1) * 128, 0:512])
            nc.scalar.dma_start(x_sb[j][:], xT[j * 128:(j + 1) * 128, :])
        nc.sync.dma_start(qb_sb[:], qbias[:])
        nc.sync.dma_start(kb_sb[:], kbias[:])
        for j in range(8):
            nc.sync.dma_start(wk_sb[j][:], wcat[j * 128:(j + 1) * 128, 512:1024])
        for j in range(8):
            nc.sync.dma_start(wv_sb[j][:], wcat[j * 128:(j + 1) * 128, 1024:1536])
        nc.vector.memset(zc[:], 0.0)
        nc.vector.memset(onec[:], 1.0)
        for pr in range(4):
            nc.sync.dma_start(pw_sb[:, pr, :], pw[pr * 128:(pr + 1) * 128, :])
            nc.vector.tensor_scalar(out=VT[pr][:, :, :, 64:65], in0=zc[:],
                                    scalar1=onec[:], scalar2=None, op0=ADD)

        def qkv_groups(pr):
            """32 emitters: (owner_pr, pe_cycles, fn) for Q(8) + K(8) + V(16)."""
            for nb in range(8):
                def gq(nb=nb):
                    acc = ps.tile([128, 512], F32, tag="acc", bufs=2)
                    for j in range(8):
                        nc.tensor.matmul(acc[:, 0:256],
                                         wq_sb[j][:, pr * 128:(pr + 1) * 128],
                                         x_sb[j][:, nb * 256:(nb + 1) * 256],
                                         start=(j == 0), stop=(j == 7))
                    nc.vector.tensor_scalar(
                        out=QT[pr][:, nb * 256:(nb + 1) * 256], in0=acc[:, 0:256],
                        scalar1=qb_sb[:, pr:pr + 1], scalar2=None, op0=ADD)
                yield pr, 2048, gq
            for nb in range(8):
                def gk(nb=nb):
                    acc = ps.tile([128, 512], F32, tag="acc", bufs=2)
                    for j in range(8):
                        nc.tensor.matmul(acc[:, 0:256],
                                         wk_sb[j][:, pr * 128:(pr + 1) * 128],
                                         x_sb[j][:, nb * 256:(nb + 1) * 256],
                                         start=(j == 0), stop=(j == 7))
                    nc.vector.tensor_scalar(
                        out=KT[pr][:, nb * 256:(nb + 1) * 256], in0=acc[:, 0:256],
                        scalar1=kb_sb[:, pr:pr + 1], scalar2=None, op0=ADD)
                yield pr, 2048, gk
            for t in range(16):
                def gv(t=t):
                    acc = ps.tile([128, 512], F32, tag="acc", bufs=2)
                    for j in range(8):
                        nc.tensor.matmul(acc[:, 0:128],
                                         x_sb[j][:, t * 128:(t + 1) * 128],
                                         wv_sb[j][:, pr * 128:(pr + 1) * 128],
                                         start=(j == 0), stop=(j == 7))
                    nc.vector.tensor_copy(
                        out=VT[pr][:, t, :, 0:64],
                        in_=acc[:, 0:128].rearrange("p (h d) -> p h d", h=2))
                yield pr, 1024, gv

        def proj_groups(qb):
            """8 emitters: output projection for query block qb."""
            q0 = qb * 512
            for ns in range(4):
                for co in range(2):
                    def gp(ns=ns, co=co):
                        pj = ps.tile([128, 512], F32, tag="acc", bufs=2)
                        for pr4 in range(4):
                            nc.tensor.matmul(
                                pj[:],
                                OT[pr4][:, q0 + ns * 128:q0 + (ns + 1) * 128],
                                pw_sb[:, pr4, co * 512:(co + 1) * 512],
                                start=(pr4 == 0), stop=(pr4 == 3))
                        so = sb.tile([128, 512], F32, tag="so", bufs=2)
                        nc.vector.tensor_copy(out=so[:], in_=pj[:])
                        nc.sync.dma_start(
                            out[q0 + ns * 128:q0 + (ns + 1) * 128,
                                co * 512:(co + 1) * 512], so[:])
                    yield 9, 2048, gp

        # Global fill queue: QKV for later head-pairs and the output
        # projection are paced evenly into the attention tg slots (~1.4k PE
        # cycles per slot) so the tensor engine always stays ahead of the
        # scalar engine's exp stream.
        fills = deque()
        st8 = {"filled": 0, "target": 0}

        def pace():
            st8["target"] += 1365
            while fills and st8["filled"] < st8["target"]:
                _, cyc, g = fills.popleft()
                g()
                st8["filled"] += cyc

        def flush(owner_max):
            while fills and fills[0][0] <= owner_max:
                _, cyc, g = fills.popleft()
                g()
                st8["filled"] += cyc

        def attn(pr, qb):
            q0 = qb * 512
            oaug0 = ps.tile([65, 512], F32, tag="oaug", bufs=2)
            oaug1 = ps.tile([65, 512], F32, tag="oaug", bufs=2)
            staged = []
            for tg in range(9):
                if tg < 8:
                    t0, t1 = 2 * tg, 2 * tg + 1
                    stage0 = ps.tile([128, 1024], F32, tag="stage", bufs=2)
                    stage1 = ps.tile([128, 1024], F32, tag="stage", bufs=2)
                    # scores S^T [keys, queries]; heads (2pr, 2pr+1) row-packed
                    nc.tensor.matmul(stage0[:, 0:512],
                                     KT[pr][0:64, t0 * 128:(t0 + 1) * 128],
                                     QT[pr][0:64, q0:q0 + 512],
                                     start=True, stop=True, tile_position=(0, 0))
                    nc.tensor.matmul(stage1[:, 0:512],
                                     KT[pr][64:128, t0 * 128:(t0 + 1) * 128],
                                     QT[pr][64:128, q0:q0 + 512],
                                     start=True, stop=True, tile_position=(64, 0))
                    nc.tensor.matmul(stage0[:, 512:1024],
                                     KT[pr][0:64, t1 * 128:(t1 + 1) * 128],
                                     QT[pr][0:64, q0:q0 + 512],
                                     start=True, stop=True, tile_position=(0, 0))
                    nc.tensor.matmul(stage1[:, 512:1024],
                                     KT[pr][64:128, t1 * 128:(t1 + 1) * 128],
                                     QT[pr][64:128, q0:q0 + 512],
                                     start=True, stop=True, tile_position=(64, 0))
                if tg >= 1:
                    # PV lags S by one tg so exp overlaps the next S pair
                    pP0, pP1, pt0, pt1 = staged[tg - 1]
                    st, sp = (tg - 1 == 0), (tg - 1 == 7)
                    nc.tensor.matmul(oaug0[:], VT[pr][:, pt0, 0, :],
                                     pP0[:, 0:512], start=st, stop=False)
                    nc.tensor.matmul(oaug0[:], VT[pr][:, pt1, 0, :],
                                     pP0[:, 512:1024], start=False, stop=sp)
                    nc.tensor.matmul(oaug1[:], VT[pr][:, pt0, 1, :],
                                     pP1[:, 0:512], start=st, stop=False)
                    nc.tensor.matmul(oaug1[:], VT[pr][:, pt1, 1, :],
                                     pP1[:, 512:1024], start=False, stop=sp)
                if tg < 8:
                    P0 = sb.tile([128, 1024], BF16, tag="p", bufs=4)
                    P1 = sb.tile([128, 1024], BF16, tag="p", bufs=4)
                    nc.scalar.activation(P0[:], stage0[:], EXP,
                                         bias=0.0, scale=SCALE)
                    nc.scalar.activation(P1[:], stage1[:], EXP,
                                         bias=0.0, scale=SCALE)
                    if dbg and pr == 0 and qb == 0 and tg == 0:
                        nc.sync.dma_start(dbgb[4:5, 0:1024], P0[0:1, :])
                        nc.sync.dma_start(dbgb[5:6, 0:1024], P1[0:1, :])
                    staged.append((P0, P1, t0, t1))
                pace()
            # normalization, off the critical path: free oaug via SBUF copy,
            # approx-reciprocal the denominator row, DRAM-bounce broadcast,
            # scale into OT.
            for hh, oaug in ((0, oaug0), (1, oaug1)):
                row = pr * 8 + qb * 2 + hh
                ost = sb.tile([65, 512], F32, tag="ost", bufs=4)
                nc.vector.tensor_copy(out=ost[:], in_=oaug[:])
                if dbg and pr == 0 and qb == 0 and hh == 0:
                    nc.sync.dma_start(dbgf[0:65, :], ost[:])
                rc = sb.tile([1, 512], F32, tag="rc", bufs=4)
                nc.vector.reciprocal(rc[:], ost[64:65, :])
                nc.sync.dma_start(scratch[row:row + 1, :], rc[:])
                rb = sb.tile([64, 512], F32, tag="rb", bufs=4)
                nc.sync.dma_start(
                    rb[:], scratch[row:row + 1, :].to_broadcast((64, 512)))
                nc.vector.tensor_tensor(out=OT[pr][hh * 64:(hh + 1) * 64,
                                                   q0:q0 + 512],
                                        in0=ost[0:64, :], in1=rb[:], op=MULT)

        for _, _, g in qkv_groups(0):
            g()
        if dbg:
            nc.sync.dma_start(dbgb[0:1, :], QT[0][0:1, :])
            nc.sync.dma_start(dbgb[1:2, :], KT[0][0:1, :])
            nc.sync.dma_start(dbgb[2:3, 0:65], VT[0][0:1, 0, 0, :])
            nc.sync.dma_start(dbgb[3:4, 0:65], VT[0][0:1, 0, 1, :])
        for p in (1, 2, 3):
            fills.extend(qkv_groups(p))
        for pr in range(4):
            flush(pr)  # QKV(pr) must be fully emitted before its attention
            for qb in range(4):
                if pr == 3 and qb >= 1:
                    fills.extend(proj_groups(qb - 1))
                attn(pr, qb)
        while fills:
            fills.popleft()[2]()
        for _, _, g in proj_groups(3):
            g()
    return nc


def _prepare_in_maps(x, qkv_w, qkv_b, proj_w):
    x = np.asarray(x, dtype=np.float32)
    wb = np.asarray(qkv_w, dtype=np.float32).astype(bfloat16)
    pwb = np.asarray(proj_w, dtype=np.float32).astype(bfloat16)
    qkv_b = np.asarray(qkv_b, dtype=np.float32)
    in_maps = []
    for c in range(8):
        b, g = c % 4, c // 4
        w0 = 512 * g
        in_maps.append({
            "xT": np.ascontiguousarray(x[b].T).astype(bfloat16),
            "wcat": np.ascontiguousarray(np.concatenate(
                [wb[:, w0:w0 + 512],
                 wb[:, 1024 + w0:1024 + w0 + 512],
                 wb[:, 2048 + w0:2048 + w0 + 512]], axis=1)),
            "qb": np.ascontiguousarray(qkv_b[w0:w0 + 512].reshape(4, 128).T),
            "kb": np.ascontiguousarray(
                qkv_b[1024 + w0:1024 + w0 + 512].reshape(4, 128).T),
            "pw": np.ascontiguousarray(pwb[w0:w0 + 512, :]),
        })
    return in_maps


def _gather(parts, qkv_b, proj_w, proj_b):
    const_row = (np.asarray(qkv_b)[2048:].astype(np.float64)
                 @ np.asarray(proj_w).astype(np.float64)
                 + np.asarray(proj_b).astype(np.float64))
    out = np.empty((B, N, C), np.float32)
    for b in range(B):
        out[b] = (parts[b].astype(np.float64) + parts[b + 4].astype(np.float64)
                  + const_row).astype(np.float32)
    return out


def kernel(**inputs: np.ndarray) -> np.ndarray:
    x = np.asarray(inputs["x"], dtype=np.float32)
    qkv_w = np.asarray(inputs["qkv_w"], dtype=np.float32)
    qkv_b = np.asarray(inputs["qkv_b"], dtype=np.float32)
    proj_w = np.asarray(inputs["proj_w"], dtype=np.float32)
    proj_b = np.asarray(inputs["proj_b"], dtype=np.float32)

    in_maps = _prepare_in_maps(x, qkv_w, qkv_b, proj_w)
    nc = _build()
    nc.finalize()
    res = run_bass_kernel_spmd(nc, in_maps, list(range(8)))
    parts = [res.results[c]["out"] for c in range(8)]
    return _gather(parts, qkv_b, proj_w, proj_b)


if __name__ == "__main__":
    import tempfile
    import time

    from concourse.bass_utils import compile_bass_kernel

    t0 = time.time()
    nc = _build()
    nc.compile()
    with tempfile.TemporaryDirectory() as td:
        compile_bass_kernel(nc, td, neff_name="k.neff")
    print(f"COMPILE OK ({time.time() - t0:.0f}s)", flush=True)


# revision 29
# speedup vs baseline: 1.2389x; 1.0048x over previous
"""Multi-head attention (B=4, N=2048, C=1024, H=16, D=64) on 8 TRN2 cores.

Sharding: core c -> batch b = c%4, head-group g = c//4 (local heads 0..7 are
global heads 8g..8g+7).  Each core computes its head group's contribution to
the output projection for its batch; host sums core b + core b+4 and adds
const_row = qkv_b[2048:] @ proj_w + proj_b (V-bias folds exactly through the
row-normalized attention: attn @ (1*bv^T) = 1*bv^T).

v2: all-bf16 datapath (inputs pre-cast on host), software-pipelined schedule:
QKV projection for head-pair pr+1 is interleaved into the attention tg-loop of
head-pair pr so the scalar engine's exp stream (the phase-2 co-bottleneck)
overlaps the tensor engine's QKV matmuls instead of idling during a separate
phase 1.  Projection for query block qb is interleaved into attention of
(pr=3, qb+1).  Softmax normalization moved off the critical path: oaug PSUM is
copied to SBUF immediately (freeing the PSUM bank for the next query block),
then reciprocal_approx_fast + DRAM-bounce broadcast + multiply trail behind.

Per-core device layout:
  x_sb[j]  [128, 2048] bf16   xT rows j*128.. (C x N), j = 0..7
  wq/wk/wv_sb[j] [128, 512]   W columns for this head group, per input-row j
  QT/KT[pr] [128, 2048] bf16  partition = dim-in-pair (2 heads x 64), free = tok
  VT[pr]   [128, 16, 2, 65]   partition = token-in-block; col 64 = ones (denom)
  OT[pr]   [128, 2048] bf16   normalized attention output, chan-pair x token
  out      [2048, 1024] f32   partial projection output
"""

import sys

sys.path.insert(0, "/opt/trn_rl_repo")

from collections import deque
from contextlib import ExitStack

import numpy as np
from ml_dtypes import bfloat16

from concourse import bacc, mybir, tile
from concourse.bass_utils import run_bass_kernel_spmd

F32 = mybir.dt.float32
BF16 = mybir.dt.bfloat16
EXP = mybir.ActivationFunctionType.Exp
ADD = mybir.AluOpType.add
MULT = mybir.AluOpType.mult
DIV = mybir.AluOpType.divide

B, N, C, H, D = 4, 2048, 1024, 16, 64
SCALE = 0.125


def _build(dbg=False):
    nc = bacc.Bacc("TRN2", target_bir_lowering=False, debug=False)
    xT = nc.dram_tensor("xT", [1024, 2048], BF16, kind="ExternalInput").ap()
    wcat = nc.dram_tensor("wcat", [1024, 1536], BF16, kind="ExternalInput").ap()
    qbias = nc.dram_tensor("qb", [128, 4], F32, kind="ExternalInput").ap()
    kbias = nc.dram_tensor("kb", [128, 4], F32, kind="ExternalInput").ap()
    pw = nc.dram_tensor("pw", [512, 1024], BF16, kind="ExternalInput").ap()
    out = nc.dram_tensor("out", [2048, 1024], F32, kind="ExternalOutput").ap()
    scratch = nc.dram_tensor("scratch", [32, 512], F32).ap()
    if dbg:
        dbgb = nc.dram_tensor("dbgb", [8, 2048], BF16,
                              kind="ExternalOutput").ap()
        dbgf = nc.dram_tensor("dbgf", [66, 512], F32,
                              kind="ExternalOutput").ap()

    with tile.TileContext(nc) as tc, ExitStack() as ctx:
        sb = ctx.enter_context(tc.tile_pool(name="sb", bufs=1))
        ps = ctx.enter_context(tc.tile_pool(name="ps", bufs=1, space="PSUM"))

        x_sb = [sb.tile([128, 2048], BF16, name=f"x{j}") for j in range(8)]
        wq_sb = [sb.tile([128, 512], BF16, name=f"wq{j}") for j in range(8)]
        wk_sb = [sb.tile([128, 512], BF16, name=f"wk{j}") for j in range(8)]
        wv_sb = [sb.tile([128, 512], BF16, name=f"wv{j}") for j in range(8)]
        pw_sb = sb.tile([128, 4, 1024], BF16, tag="pw")
        QT = [sb.tile([128, 2048], BF16, name=f"QT{p}") for p in range(4)]
        KT = [sb.tile([128, 2048], BF16, name=f"KT{p}") for p in range(4)]
        VT = [sb.tile([128, 16, 2, 65], BF16, name=f"VT{p}") for p in range(4)]
        OT = [sb.tile([128, 2048], BF16, name=f"OT{p}") for p in range(4)]
        qb_sb = sb.tile([128, 4], F32, tag="qb")
        kb_sb = sb.tile([128, 4], F32, tag="kb")
        zc = sb.tile([128, 16, 2, 1], F32, tag="zc")
        onec = sb.tile([128, 1], F32, tag="onec")

        # loads ordered by first use: Q weights + x stream, then K, V, proj.
        # x goes out on the scalar engine's DMA queue, weights on sync's, so
        # the two streams transfer in parallel and the first QKV matmul can
        # start ~8us earlier.
        for j in range(8):
            nc.sync.dma_start(wq_sb[j][:], wcat[j * 128:(j + 1) * 128, 0:512])
            nc.scalar.dma_start(x_sb[j][:], xT[j * 128:(j + 1) * 128, :])
        nc.sync.dma_start(qb_sb[:], qbias[:])
        nc.sync.dma_start(kb_sb[:], kbias[:])
        for j in range(8):
            nc.sync.dma_start(wk_sb[j][:], wcat[j * 128:(j + 1) * 128, 512:1024])
        for j in range(8):
            nc.sync.dma_start(wv_sb[j][:], wcat[j * 128:(j + 1) * 128, 1024:1536])
        nc.vector.memset(zc[:], 0.0)
        nc.vector.memset(onec[:], 1.0)
        for pr in range(4):
            nc.sync.dma_start(pw_sb[:, pr, :], pw[pr * 128:(pr + 1) * 128, :])
            nc.vector.tensor_scalar(out=VT[pr][:, :, :, 64:65], in0=zc[:],
                                    scalar1=onec[:], scalar2=None, op0=ADD)

        def qkv_groups(pr):
            """32 emitters: (owner_pr, pe_cycles, fn) for Q(8) + K(8) + V(16)."""
            for nb in range(8):
                def gq(nb=nb):
                    acc = ps.tile([128, 512], F32, tag="acc", bufs=2)
                    for j in range(8):
                        nc.tensor.matmul(acc[:, 0:256],
                                         wq_sb[j][:, pr * 128:(pr + 1) * 128],
                                         x_sb[j][:, nb * 256:(nb + 1) * 256],
                                         start=(j == 0), stop=(j == 7))
                    nc.vector.tensor_scalar(
                        out=QT[pr][:, nb * 256:(nb + 1) * 256], in0=acc[:, 0:256],
                        scalar1=qb_sb[:, pr:pr + 1], scalar2=None, op0=ADD)
                yield pr, 2048, gq
            for nb in range(8):
                def gk(nb=nb):
                    acc = ps.tile([128, 512], F32, tag="acc", bufs=2)
                    for j in range(8):
                        nc.tensor.matmul(acc[:, 0:256],
                                         wk_sb[j][:, pr * 128:(pr + 1) * 128],
                                         x_sb[j][:, nb * 256:(nb + 1) * 256],
                                         start=(j == 0), stop=(j == 7))
                    nc.vector.tensor_scalar(
                        out=KT[pr][:, nb * 256:(nb + 1) * 256], in0=acc[:, 0:256],
                        scalar1=kb_sb[:, pr:pr + 1], scalar2=None, op0=ADD)
                yield pr, 2048, gk
            for t in range(16):
                def gv(t=t):
                    acc = ps.tile([128, 512], F32, tag="acc", bufs=2)
                    for j in range(8):
                        nc.tensor.matmul(acc[:, 0:128],
                                         x_sb[j][:, t * 128:(t + 1) * 128],
                                         wv_sb[j][:, pr * 128:(pr + 1) * 128],
                                         start=(j == 0), stop=(j == 7))
                    nc.vector.tensor_copy(
                        out=VT[pr][:, t, :, 0:64],
                        in_=acc[:, 0:128].rearrange("p (h d) -> p h d", h=2))
                yield pr, 1024, gv

        def proj_groups(qb):
            """8 emitters: output projection for query block qb."""
            q0 = qb * 512
            for ns in range(4):
                for co in range(2):
                    def gp(ns=ns, co=co):
                        pj = ps.tile([128, 512], F32, tag="acc", bufs=2)
                        for pr4 in range(4):
                            nc.tensor.matmul(
                                pj[:],
                                OT[pr4][:, q0 + ns * 128:q0 + (ns + 1) * 128],
                                pw_sb[:, pr4, co * 512:(co + 1) * 512],
                                start=(pr4 == 0), stop=(pr4 == 3))
                        so = sb.tile([128, 512], F32, tag="so", bufs=2)
                        nc.vector.tensor_copy(out=so[:], in_=pj[:])
                        nc.sync.dma_start(
                            out[q0 + ns * 128:q0 + (ns + 1) * 128,
                                co * 512:(co + 1) * 512], so[:])
                    yield 9, 2048, gp

        # Global fill queue: QKV for later head-pairs and the output
        # projection are paced evenly into the attention tg slots (~1.4k PE
        # cycles per slot) so the tensor engine always stays ahead of the
        # scalar engine's exp stream.
        fills = deque()
        pending = deque()  # deferred normalize tails (DVE divide)
        st8 = {"filled": 0, "target": 0}

        def pace():
            st8["target"] += 1365
            while fills and st8["filled"] < st8["target"]:
                _, cyc, g = fills.popleft()
                g()
                st8["filled"] += cyc

        def flush(owner_max):
            while fills and fills[0][0] <= owner_max:
                _, cyc, g = fills.popleft()
                g()
                st8["filled"] += cyc

        def attn(pr, qb, late=None):
            q0 = qb * 512
            oaug0 = ps.tile([65, 512], F32, tag="oaug", bufs=2)
            oaug1 = ps.tile([65, 512], F32, tag="oaug", bufs=2)
            staged = []
            for tg in range(9):
                if tg < 8:
                    t0, t1 = 2 * tg, 2 * tg + 1
                    stage0 = ps.tile([128, 1024], F32, tag="stage", bufs=2)
                    stage1 = ps.tile([128, 1024], F32, tag="stage", bufs=2)
                    # scores S^T [keys, queries]; heads (2pr, 2pr+1) row-packed
                    nc.tensor.matmul(stage0[:, 0:512],
                                     KT[pr][0:64, t0 * 128:(t0 + 1) * 128],
                                     QT[pr][0:64, q0:q0 + 512],
                                     start=True, stop=True, tile_position=(0, 0))
                    nc.tensor.matmul(stage1[:, 0:512],
                                     KT[pr][64:128, t0 * 128:(t0 + 1) * 128],
                                     QT[pr][64:128, q0:q0 + 512],
                                     start=True, stop=True, tile_position=(64, 0))
                    nc.tensor.matmul(stage0[:, 512:1024],
                                     KT[pr][0:64, t1 * 128:(t1 + 1) * 128],
                                     QT[pr][0:64, q0:q0 + 512],
                                     start=True, stop=True, tile_position=(0, 0))
                    nc.tensor.matmul(stage1[:, 512:1024],
                                     KT[pr][64:128, t1 * 128:(t1 + 1) * 128],
                                     QT[pr][64:128, q0:q0 + 512],
                                     start=True, stop=True, tile_position=(64, 0))
                if tg >= 1:
                    # PV lags S by one tg so exp overlaps the next S pair
                    pP0, pP1, pt0, pt1 = staged[tg - 1]
                    st, sp = (tg - 1 == 0), (tg - 1 == 7)
                    nc.tensor.matmul(oaug0[:], VT[pr][:, pt0, 0, :],
                                     pP0[:, 0:512], start=st, stop=False)
                    nc.tensor.matmul(oaug0[:], VT[pr][:, pt1, 0, :],
                                     pP0[:, 512:1024], start=False, stop=sp)
                    nc.tensor.matmul(oaug1[:], VT[pr][:, pt0, 1, :],
                                     pP1[:, 0:512], start=st, stop=False)
                    nc.tensor.matmul(oaug1[:], VT[pr][:, pt1, 1, :],
                                     pP1[:, 512:1024], start=False, stop=sp)
                if tg < 8:
                    P0 = sb.tile([128, 1024], BF16, tag="p", bufs=4)
                    P1 = sb.tile([128, 1024], BF16, tag="p", bufs=4)
                    nc.scalar.activation(P0[:], stage0[:], EXP,
                                         bias=0.0, scale=SCALE)
                    nc.scalar.activation(P1[:], stage1[:], EXP,
                                         bias=0.0, scale=SCALE)
                    if dbg and pr == 0 and qb == 0 and tg == 0:
                        nc.sync.dma_start(dbgb[4:5, 0:1024], P0[0:1, :])
                        nc.sync.dma_start(dbgb[5:6, 0:1024], P1[0:1, :])
                    staged.append((P0, P1, t0, t1))
                if pending:
                    pending.popleft()()
                if tg == 8 and late is not None:
                    fills.extend(late)
                pace()
            # normalization, off the critical path: free oaug via SBUF copy,
            # approx-reciprocal the denominator row, DRAM-bounce broadcast,
            # scale into OT.
            # Normalize tail, deferred: oaug is freed via an immediate SBUF
            # copy; the serial DVE reciprocals are split into [1,256] halves
            # and drip-fed (one piece per tg slot) into the NEXT attention
            # block so they never delay the PSUM-recycling DVE ops that gate
            # the tensor engine.
            osts, rcs, rbs = [], [], []
            for hh, oaug in ((0, oaug0), (1, oaug1)):
                ost = sb.tile([65, 512], F32, tag="ost", bufs=4)
                nc.vector.tensor_copy(out=ost[:], in_=oaug[:])
                if dbg and pr == 0 and qb == 0 and hh == 0:
                    nc.sync.dma_start(dbgf[0:65, :], ost[:])
                osts.append(ost)
                rcs.append(sb.tile([1, 512], F32, tag="rc", bufs=4, name="rc"))
                rbs.append(sb.tile([64, 512], F32, tag="rb", bufs=4, name="rb"))

            def recip_piece(hh, half, pr=pr, qb=qb, osts=osts, rcs=rcs):
                row = pr * 8 + qb * 2 + hh
                lo, hi = half * 256, (half + 1) * 256
                nc.vector.reciprocal(rcs[hh][0:1, lo:hi], osts[hh][64:65, lo:hi])
                nc.sync.dma_start(scratch[row:row + 1, lo:hi],
                                  rcs[hh][0:1, lo:hi])

            def bcast_piece(pr=pr, qb=qb, rbs=rbs):
                for hh in range(2):
                    row = pr * 8 + qb * 2 + hh
                    nc.sync.dma_start(
                        rbs[hh][:],
                        scratch[row:row + 1, :].to_broadcast((64, 512)))

            def norm_piece(hh, pr=pr, q0=q0, osts=osts, rbs=rbs):
                nc.vector.tensor_tensor(out=OT[pr][hh * 64:(hh + 1) * 64,
                                                   q0:q0 + 512],
                                        in0=osts[hh][0:64, :], in1=rbs[hh][:],
                                        op=MULT)

            pending.extend([
                lambda: recip_piece(0, 0), lambda: recip_piece(1, 0),
                lambda: recip_piece(0, 1), lambda: recip_piece(1, 1),
                bcast_piece,
                lambda: norm_piece(0), lambda: norm_piece(1),
            ])

        for _, _, g in qkv_groups(0):
            g()
        if dbg:
            nc.sync.dma_start(dbgb[0:1, :], QT[0][0:1, :])
            nc.sync.dma_start(dbgb[1:2, :], KT[0][0:1, :])
            nc.sync.dma_start(dbgb[2:3, 0:65], VT[0][0:1, 0, 0, :])
            nc.sync.dma_start(dbgb[3:4, 0:65], VT[0][0:1, 0, 1, :])
        for p in (1, 2, 3):
            fills.extend(qkv_groups(p))
        for pr in range(4):
            flush(pr)  # QKV(pr) must be fully emitted before its attention
            for qb in range(4):
                late = proj_groups(qb - 1) if pr == 3 and qb >= 1 else None
                attn(pr, qb, late=late)
        while pending:
            pending.popleft()()
        while fills:
            fills.popleft()[2]()
        for _, _, g in proj_groups(3):
            g()
    return nc


def _prepare_in_maps(x, qkv_w, qkv_b, proj_w):
    x = np.asarray(x, dtype=np.float32)
    wb = np.asarray(qkv_w, dtype=np.float32).astype(bfloat16)
    pwb = np.asarray(proj_w, dtype=np.float32).astype(bfloat16)
    qkv_b = np.asarray(qkv_b, dtype=np.float32)
    in_maps = []
    for c in range(8):
        b, g = c % 4, c // 4
        w0 = 512 * g
        in_maps.append({
            "xT": np.ascontiguousarray(x[b].T).astype(bfloat16),
            "wcat": np.ascontiguousarray(np.concatenate(
                [wb[:, w0:w0 + 512],
                 wb[:, 1024 + w0:1024 + w0 + 512],
                 wb[:, 2048 + w0:2048 + w0 + 512]], axis=1)),
            "qb": np.ascontiguousarray(qkv_b[w0:w0 + 512].reshape(4, 128).T),
            "kb": np.ascontiguousarray(
                qkv_b[1024 + w0:1024 + w0 + 512].reshape(4, 128).T),
            "pw": np.ascontiguousarray(pwb[w0:w0 + 512, :]),
        })
    return in_maps


def _gather(parts, qkv_b, proj_w, proj_b):
    const_row = (np.asarray(qkv_b)[2048:].astype(np.float64)
                 @ np.asarray(proj_w).astype(np.float64)
                 + np.asarray(proj_b).astype(np.float64))
    out = np.empty((B, N, C), np.float32)
    for b in range(B):
        out[b] = (parts[b].astype(np.float64) + parts[b + 4].astype(np.float64)
                  + const_row).astype(np.float32)
    return out


def kernel(**inputs: np.ndarray) -> np.ndarray:
    x = np.asarray(inputs["x"], dtype=np.float32)
    qkv_w = np.asarray(inputs["qkv_w"], dtype=np.float32)
    qkv_b = np.asarray(inputs["qkv_b"], dtype=np.float32)
    proj_w = np.asarray(inputs["proj_w"], dtype=np.float32)
    proj_b = np.asarray(inputs["proj_b"], dtype=np.float32)

    in_maps = _prepare_in_maps(x, qkv_w, qkv_b, proj_w)
    nc = _build()
    nc.finalize()
    res = run_bass_kernel_spmd(nc, in_maps, list(range(8)))
    parts = [res.results[c]["out"] for c in range(8)]
    return _gather(parts, qkv_b, proj_w, proj_b)


if __name__ == "__main__":
    import tempfile
    import time

    from concourse.bass_utils import compile_bass_kernel

    t0 = time.time()
    nc = _build()
    nc.compile()
    with tempfile.TemporaryDirectory() as td:
        compile_bass_kernel(nc, td, neff_name="k.neff")
    print(f"COMPILE OK ({time.time() - t0:.0f}s)", flush=True)


# revision 35
# speedup vs baseline: 1.2456x; 1.0054x over previous
"""Multi-head attention (B=4, N=2048, C=1024, H=16, D=64) on 8 TRN2 cores.

Sharding: core c -> batch b = c%4, head-group g = c//4 (local heads 0..7 are
global heads 8g..8g+7).  Each core computes its head group's contribution to
the output projection for its batch; host sums core b + core b+4 and adds
const_row = qkv_b[2048:] @ proj_w + proj_b (V-bias folds exactly through the
row-normalized attention: attn @ (1*bv^T) = 1*bv^T).

v2: all-bf16 datapath (inputs pre-cast on host), software-pipelined schedule:
QKV projection for head-pair pr+1 is interleaved into the attention tg-loop of
head-pair pr so the scalar engine's exp stream (the phase-2 co-bottleneck)
overlaps the tensor engine's QKV matmuls instead of idling during a separate
phase 1.  Projection for query block qb is interleaved into attention of
(pr=3, qb+1).  Softmax normalization moved off the critical path: oaug PSUM is
copied to SBUF immediately (freeing the PSUM bank for the next query block),
then reciprocal_approx_fast + DRAM-bounce broadcast + multiply trail behind.

Per-core device layout:
  x_sb[j]  [128, 2048] bf16   xT rows j*128.. (C x N), j = 0..7
  wq/wk/wv_sb[j] [128, 512]   W columns for this head group, per input-row j
  QT/KT[pr] [128, 2048] bf16  partition = dim-in-pair (2 heads x 64), free = tok
  VT[pr]   [128, 16, 2, 65]   partition = token-in-block; col 64 = ones (denom)
  OT[pr]   [128, 2048] bf16   normalized attention output, chan-pair x token
  out      [2048, 1024] f32   partial projection output
"""

import sys

sys.path.insert(0, "/opt/trn_rl_repo")

from collections import deque
from contextlib import ExitStack

import numpy as np
from ml_dtypes import bfloat16

from concourse import bacc, mybir, tile
from concourse.bass_utils import run_bass_kernel_spmd

F32 = mybir.dt.float32
BF16 = mybir.dt.bfloat16
EXP = mybir.ActivationFunctionType.Exp
ADD = mybir.AluOpType.add
MULT = mybir.AluOpType.mult
DIV = mybir.AluOpType.divide

B, N, C, H, D = 4, 2048, 1024, 16, 64
SCALE = 0.125


def _build(dbg=False):
    nc = bacc.Bacc("TRN2", target_bir_lowering=False, debug=False)
    xT = nc.dram_tensor("xT", [1024, 2048], BF16, kind="ExternalInput").ap()
    wcat = nc.dram_tensor("wcat", [1024, 1536], BF16, kind="ExternalInput").ap()
    qbias = nc.dram_tensor("qb", [128, 4], F32, kind="ExternalInput").ap()
    kbias = nc.dram_tensor("kb", [128, 4], F32, kind="ExternalInput").ap()
    pw = nc.dram_tensor("pw", [512, 1024], BF16, kind="ExternalInput").ap()
    out = nc.dram_tensor("out", [2048, 1024], F32, kind="ExternalOutput").ap()
    scratch = nc.dram_tensor("scratch", [32, 512], F32).ap()
    if dbg:
        dbgb = nc.dram_tensor("dbgb", [8, 2048], BF16,
                              kind="ExternalOutput").ap()
        dbgf = nc.dram_tensor("dbgf", [66, 512], F32,
                              kind="ExternalOutput").ap()

    with tile.TileContext(nc) as tc, ExitStack() as ctx:
        sb = ctx.enter_context(tc.tile_pool(name="sb", bufs=1))
        ps = ctx.enter_context(tc.tile_pool(name="ps", bufs=1, space="PSUM"))

        x_sb = [sb.tile([128, 2048], BF16, name=f"x{j}") for j in range(8)]
        wq_sb = [sb.tile([128, 512], BF16, name=f"wq{j}") for j in range(8)]
        wk_sb = [sb.tile([128, 512], BF16, name=f"wk{j}") for j in range(8)]
        wv_sb = [sb.tile([128, 512], BF16, name=f"wv{j}") for j in range(8)]
        pw_sb = sb.tile([128, 4, 1024], BF16, tag="pw")
        QT = [sb.tile([128, 2048], BF16, name=f"QT{p}") for p in range(4)]
        KT = [sb.tile([128, 2048], BF16, name=f"KT{p}") for p in range(4)]
        VT = [sb.tile([128, 16, 2, 65], BF16, name=f"VT{p}") for p in range(4)]
        OT = [sb.tile([128, 2048], BF16, name=f"OT{p}") for p in range(4)]
        qb_sb = sb.tile([128, 4], F32, tag="qb")
        kb_sb = sb.tile([128, 4], F32, tag="kb")
        zc = sb.tile([128, 16, 2, 1], F32, tag="zc")
        onec = sb.tile([128, 1], F32, tag="onec")

        # loads ordered by first use: Q weights + x stream, then K, V, proj.
        # x goes out on the scalar engine's DMA queue, weights on sync's, so
        # the two streams transfer in parallel and the first QKV matmul can
        # start ~8us earlier.
        for j in range(8):
            nc.sync.dma_start(wq_sb[j][:], wcat[j * 128:(j + 1) * 128, 0:512])
            nc.sync.dma_start(x_sb[j][:], xT[j * 128:(j + 1) * 128, :])
        nc.sync.dma_start(qb_sb[:], qbias[:])
        nc.sync.dma_start(kb_sb[:], kbias[:])
        for j in range(8):
            nc.sync.dma_start(wk_sb[j][:], wcat[j * 128:(j + 1) * 128, 512:1024])
        for j in range(8):
            nc.sync.dma_start(wv_sb[j][:], wcat[j * 128:(j + 1) * 128, 1024:1536])
        nc.vector.memset(zc[:], 0.0)
        nc.vector.memset(onec[:], 1.0)
        for pr in range(4):
            nc.sync.dma_start(pw_sb[:, pr, :], pw[pr * 128:(pr + 1) * 128, :])
            nc.vector.tensor_scalar(out=VT[pr][:, :, :, 64:65], in0=zc[:],
                                    scalar1=onec[:], scalar2=None, op0=ADD)

        def qkv_groups(pr):
            """32 emitters: (owner_pr, pe_cycles, fn) for Q(8) + K(8) + V(16)."""
            for nb in range(8):
                def gq(nb=nb):
                    acc = ps.tile([128, 512], F32, tag="acc", bufs=2)
                    for j in range(8):
                        nc.tensor.matmul(acc[:, 0:256],
                                         wq_sb[j][:, pr * 128:(pr + 1) * 128],
                                         x_sb[j][:, nb * 256:(nb + 1) * 256],
                                         start=(j == 0), stop=(j == 7))
                    nc.vector.tensor_scalar(
                        out=QT[pr][:, nb * 256:(nb + 1) * 256], in0=acc[:, 0:256],
                        scalar1=qb_sb[:, pr:pr + 1], scalar2=None, op0=ADD)
                yield pr, 2048, gq
            for nb in range(8):
                def gk(nb=nb):
                    acc = ps.tile([128, 512], F32, tag="acc", bufs=2)
                    for j in range(8):
                        nc.tensor.matmul(acc[:, 0:256],
                                         wk_sb[j][:, pr * 128:(pr + 1) * 128],
                                         x_sb[j][:, nb * 256:(nb + 1) * 256],
                                         start=(j == 0), stop=(j == 7))
                    nc.vector.tensor_scalar(
                        out=KT[pr][:, nb * 256:(nb + 1) * 256], in0=acc[:, 0:256],
                        scalar1=kb_sb[:, pr:pr + 1], scalar2=None, op0=ADD)
                yield pr, 2048, gk
            for t in range(16):
                def gv(t=t):
                    acc = ps.tile([128, 512], F32, tag="acc", bufs=2)
                    for j in range(8):
                        nc.tensor.matmul(acc[:, 0:128],
                                         x_sb[j][:, t * 128:(t + 1) * 128],
                                         wv_sb[j][:, pr * 128:(pr + 1) * 128],
                                         start=(j == 0), stop=(j == 7))
                    nc.vector.tensor_copy(
                        out=VT[pr][:, t, :, 0:64],
                        in_=acc[:, 0:128].rearrange("p (h d) -> p h d", h=2))
                yield pr, 1024, gv

        def proj_groups(qb):
            """8 emitters: output projection for query block qb."""
            q0 = qb * 512
            for ns in range(4):
                for co in range(2):
                    def gp(ns=ns, co=co):
                        pj = ps.tile([128, 512], F32, tag="acc", bufs=2)
                        for pr4 in range(4):
                            nc.tensor.matmul(
                                pj[:],
                                OT[pr4][:, q0 + ns * 128:q0 + (ns + 1) * 128],
                                pw_sb[:, pr4, co * 512:(co + 1) * 512],
                                start=(pr4 == 0), stop=(pr4 == 3))
                        so = sb.tile([128, 512], F32, tag="so", bufs=2)
                        nc.vector.tensor_copy(out=so[:], in_=pj[:])
                        nc.sync.dma_start(
                            out[q0 + ns * 128:q0 + (ns + 1) * 128,
                                co * 512:(co + 1) * 512], so[:])
                    yield 9, 2048, gp

        # Global fill queue: QKV for later head-pairs and the output
        # projection are paced evenly into the attention tg slots (~1.4k PE
        # cycles per slot) so the tensor engine always stays ahead of the
        # scalar engine's exp stream.
        fills = deque()
        pending = deque()  # deferred normalize tails (DVE divide)
        st8 = {"filled": 0, "target": 0}

        def pace():
            st8["target"] += 1365
            popped = 0
            while fills and st8["filled"] < st8["target"] and popped < 2:
                _, cyc, g = fills.popleft()
                g()
                st8["filled"] += cyc
                popped += 1

        def flush(owner_max):
            while fills and fills[0][0] <= owner_max:
                _, cyc, g = fills.popleft()
                g()
                st8["filled"] += cyc

        def attn(pr, qb, late=None):
            q0 = qb * 512
            oaug0 = ps.tile([65, 512], F32, tag="oaug", bufs=2)
            oaug1 = ps.tile([65, 512], F32, tag="oaug", bufs=2)
            staged = []
            for tg in range(9):
                if tg < 8:
                    t0, t1 = 2 * tg, 2 * tg + 1
                    stage0 = ps.tile([128, 1024], F32, tag="stage", bufs=2)
                    stage1 = ps.tile([128, 1024], F32, tag="stage", bufs=2)
                    # scores S^T [keys, queries]; heads (2pr, 2pr+1) row-packed
                    nc.tensor.matmul(stage0[:, 0:512],
                                     KT[pr][0:64, t0 * 128:(t0 + 1) * 128],
                                     QT[pr][0:64, q0:q0 + 512],
                                     start=True, stop=True, tile_position=(0, 0))
                    nc.tensor.matmul(stage1[:, 0:512],
                                     KT[pr][64:128, t0 * 128:(t0 + 1) * 128],
                                     QT[pr][64:128, q0:q0 + 512],
                                     start=True, stop=True, tile_position=(64, 0))
                    nc.tensor.matmul(stage0[:, 512:1024],
                                     KT[pr][0:64, t1 * 128:(t1 + 1) * 128],
                                     QT[pr][0:64, q0:q0 + 512],
                                     start=True, stop=True, tile_position=(0, 0))
                    nc.tensor.matmul(stage1[:, 512:1024],
                                     KT[pr][64:128, t1 * 128:(t1 + 1) * 128],
                                     QT[pr][64:128, q0:q0 + 512],
                                     start=True, stop=True, tile_position=(64, 0))
                if tg >= 1:
                    # PV lags S by one tg so exp overlaps the next S pair
                    pP0, pP1, pt0, pt1 = staged[tg - 1]
                    st, sp = (tg - 1 == 0), (tg - 1 == 7)
                    nc.tensor.matmul(oaug0[:], VT[pr][:, pt0, 0, :],
                                     pP0[:, 0:512], start=st, stop=False)
                    nc.tensor.matmul(oaug0[:], VT[pr][:, pt1, 0, :],
                                     pP0[:, 512:1024], start=False, stop=sp)
                    nc.tensor.matmul(oaug1[:], VT[pr][:, pt0, 1, :],
                                     pP1[:, 0:512], start=st, stop=False)
                    nc.tensor.matmul(oaug1[:], VT[pr][:, pt1, 1, :],
                                     pP1[:, 512:1024], start=False, stop=sp)
                if tg < 8:
                    P0 = sb.tile([128, 1024], BF16, tag="p", bufs=4)
                    P1 = sb.tile([128, 1024], BF16, tag="p", bufs=4)
                    nc.scalar.activation(P0[:], stage0[:], EXP,
                                         bias=0.0, scale=SCALE)
                    nc.scalar.activation(P1[:], stage1[:], EXP,
                                         bias=0.0, scale=SCALE)
                    if dbg and pr == 0 and qb == 0 and tg == 0:
                        nc.sync.dma_start(dbgb[4:5, 0:1024], P0[0:1, :])
                        nc.sync.dma_start(dbgb[5:6, 0:1024], P1[0:1, :])
                    staged.append((P0, P1, t0, t1))
                if pending:
                    pending.popleft()()
                if tg == 7 and late is not None:
                    fills.extend(late)
                pace()
            # normalization, off the critical path: free oaug via SBUF copy,
            # approx-reciprocal the denominator row, DRAM-bounce broadcast,
            # scale into OT.
            # Normalize tail, deferred into the next attention block: oaug is
            # freed via an immediate DVE copy; the serial DVE reciprocal is
            # split into [1,256] halves dripped one per tg slot so the DVE
            # never builds a backlog ahead of the PSUM-recycling ops that gate
            # the tensor engine; the final row-broadcast multiply runs on the
            # otherwise-idle GPSIMD/Pool engine.
            osts, rcs, rbs = [], [], []
            for hh, oaug in ((0, oaug0), (1, oaug1)):
                ost = sb.tile([65, 512], F32, tag="ost", bufs=4)
                nc.vector.tensor_copy(out=ost[:], in_=oaug[:])
                if dbg and pr == 0 and qb == 0 and hh == 0:
                    nc.sync.dma_start(dbgf[0:65, :], ost[:])
                osts.append(ost)
                rcs.append(sb.tile([1, 512], F32, tag="rc", bufs=4, name="rc"))
                rbs.append(sb.tile([64, 512], F32, tag="rb", bufs=4, name="rb"))

            def recip_piece(hh, half, pr=pr, qb=qb, osts=osts, rcs=rcs):
                row = pr * 8 + qb * 2 + hh
                lo, hi = half * 256, (half + 1) * 256
                nc.vector.reciprocal(rcs[hh][0:1, lo:hi], osts[hh][64:65, lo:hi])
                nc.sync.dma_start(scratch[row:row + 1, lo:hi],
                                  rcs[hh][0:1, lo:hi])

            def bcast_piece(pr=pr, qb=qb, rbs=rbs):
                for hh in range(2):
                    row = pr * 8 + qb * 2 + hh
                    nc.sync.dma_start(
                        rbs[hh][:],
                        scratch[row:row + 1, :].to_broadcast((64, 512)))

            def norm_piece(hh, pr=pr, q0=q0, osts=osts, rbs=rbs):
                nc.gpsimd.tensor_tensor(out=OT[pr][hh * 64:(hh + 1) * 64,
                                                   q0:q0 + 512],
                                        in0=osts[hh][0:64, :], in1=rbs[hh][:],
                                        op=MULT)

            pending.extend([
                lambda: recip_piece(0, 0), lambda: recip_piece(1, 0),
                lambda: recip_piece(0, 1), lambda: recip_piece(1, 1),
                bcast_piece,
                lambda: norm_piece(0), lambda: norm_piece(1),
            ])

        for _, _, g in qkv_groups(0):
            g()
        if dbg:
            nc.sync.dma_start(dbgb[0:1, :], QT[0][0:1, :])
            nc.sync.dma_start(dbgb[1:2, :], KT[0][0:1, :])
            nc.sync.dma_start(dbgb[2:3, 0:65], VT[0][0:1, 0, 0, :])
            nc.sync.dma_start(dbgb[3:4, 0:65], VT[0][0:1, 0, 1, :])
        for p in (1, 2, 3):
            fills.extend(qkv_groups(p))
        for pr in range(4):
            flush(pr)  # QKV(pr) must be fully emitted before its attention
            for qb in range(4):
                late = proj_groups(qb - 1) if pr == 3 and qb >= 1 else None
                attn(pr, qb, late=late)
        while pending:
            pending.popleft()()
        while fills:
            fills.popleft()[2]()
        for _, _, g in proj_groups(3):
            g()
    return nc


def _prepare_in_maps(x, qkv_w, qkv_b, proj_w):
    x = np.asarray(x, dtype=np.float32)
    wb = np.asarray(qkv_w, dtype=np.float32).astype(bfloat16)
    pwb = np.asarray(proj_w, dtype=np.float32).astype(bfloat16)
    qkv_b = np.asarray(qkv_b, dtype=np.float32)
    in_maps = []
    for c in range(8):
        b, g = c % 4, c // 4
        w0 = 512 * g
        in_maps.append({
            "xT": np.ascontiguousarray(x[b].T).astype(bfloat16),
            "wcat": np.ascontiguousarray(np.concatenate(
                [wb[:, w0:w0 + 512],
                 wb[:, 1024 + w0:1024 + w0 + 512],
                 wb[:, 2048 + w0:2048 + w0 + 512]], axis=1)),
            "qb": np.ascontiguousarray(qkv_b[w0:w0 + 512].reshape(4, 128).T),
            "kb": np.ascontiguousarray(
                qkv_b[1024 + w0:1024 + w0 + 512].reshape(4, 128).T),
            "pw": np.ascontiguousarray(pwb[w0:w0 + 512, :]),
        })
    return in_maps


def _gather(parts, qkv_b, proj_w, proj_b):
    const_row = (np.asarray(qkv_b)[2048:].astype(np.float64)
                 @ np.asarray(proj_w).astype(np.float64)
                 + np.asarray(proj_b).astype(np.float64))
    out = np.empty((B, N, C), np.float32)
    for b in range(B):
        out[b] = (parts[b].astype(np.float64) + parts[b + 4].astype(np.float64)
                  + const_row).astype(np.float32)
    return out


def kernel(**inputs: np.ndarray) -> np.ndarray:
    x = np.asarray(inputs["x"], dtype=np.float32)
    qkv_w = np.asarray(inputs["qkv_w"], dtype=np.float32)
    qkv_b = np.asarray(inputs["qkv_b"], dtype=np.float32)
    proj_w = np.asarray(inputs["proj_w"], dtype=np.float32)
    proj_b = np.asarray(inputs["proj_b"], dtype=np.float32)

    in_maps = _prepare_in_maps(x, qkv_w, qkv_b, proj_w)
    nc = _build()
    nc.finalize()
    res = run_bass_kernel_spmd(nc, in_maps, list(range(8)))
    parts = [res.results[c]["out"] for c in range(8)]
    return _gather(parts, qkv_b, proj_w, proj_b)


if __name__ == "__main__":
    import tempfile
    import time

    from concourse.bass_utils import compile_bass_kernel

    t0 = time.time()
    nc = _build()
    nc.compile()
    with tempfile.TemporaryDirectory() as td:
        compile_bass_kernel(nc, td, neff_name="k.neff")
    print(f"COMPILE OK ({time.time() - t0:.0f}s)", flush=True)


# revision 40
# speedup vs baseline: 1.2707x; 1.0201x over previous
"""Multi-head attention (B=4, N=2048, C=1024, H=16, D=64) on 8 TRN2 cores.

Sharding: core c -> batch b = c%4, head-group g = c//4 (local heads 0..7 are
global heads 8g..8g+7).  Each core computes its head group's contribution to
the output projection for its batch; host sums core b + core b+4 and adds
const_row = qkv_b[2048:] @ proj_w + proj_b (V-bias folds exactly through the
row-normalized attention: attn @ (1*bv^T) = 1*bv^T).

v2: all-bf16 datapath (inputs pre-cast on host), software-pipelined schedule:
QKV projection for head-pair pr+1 is interleaved into the attention tg-loop of
head-pair pr so the scalar engine's exp stream (the phase-2 co-bottleneck)
overlaps the tensor engine's QKV matmuls instead of idling during a separate
phase 1.  Projection for query block qb is interleaved into attention of
(pr=3, qb+1).  Softmax normalization moved off the critical path: oaug PSUM is
copied to SBUF immediately (freeing the PSUM bank for the next query block),
then reciprocal_approx_fast + DRAM-bounce broadcast + multiply trail behind.

Per-core device layout:
  x_sb[j]  [128, 2048] bf16   xT rows j*128.. (C x N), j = 0..7
  wq/wk/wv_sb[j] [128, 512]   W columns for this head group, per input-row j
  QT/KT[pr] [128, 2048] bf16  partition = dim-in-pair (2 heads x 64), free = tok
  VT[pr]   [128, 16, 2, 65]   partition = token-in-block; col 64 = ones (denom)
  OT[pr]   [128, 2048] bf16   normalized attention output, chan-pair x token
  out      [2048, 1024] f32   partial projection output
"""

import sys

sys.path.insert(0, "/opt/trn_rl_repo")

from collections import deque
from contextlib import ExitStack

import numpy as np
from ml_dtypes import bfloat16

from concourse import bacc, mybir, tile
from concourse.bass_utils import run_bass_kernel_spmd

F32 = mybir.dt.float32
BF16 = mybir.dt.bfloat16
EXP = mybir.ActivationFunctionType.Exp
ADD = mybir.AluOpType.add
MULT = mybir.AluOpType.mult
DIV = mybir.AluOpType.divide

B, N, C, H, D = 4, 2048, 1024, 16, 64
SCALE = 0.125


def _build(dbg=False):
    nc = bacc.Bacc("TRN2", target_bir_lowering=False, debug=False)
    xT = nc.dram_tensor("xT", [1024, 2048], BF16, kind="ExternalInput").ap()
    wcat = nc.dram_tensor("wcat", [1024, 1536], BF16, kind="ExternalInput").ap()
    qbias = nc.dram_tensor("qb", [128, 4], F32, kind="ExternalInput").ap()
    kbias = nc.dram_tensor("kb", [128, 4], F32, kind="ExternalInput").ap()
    pw = nc.dram_tensor("pw", [512, 1024], BF16, kind="ExternalInput").ap()
    out = nc.dram_tensor("out", [2048, 1024], F32, kind="ExternalOutput").ap()
    scratch = nc.dram_tensor("scratch", [32, 512], F32).ap()
    if dbg:
        dbgb = nc.dram_tensor("dbgb", [8, 2048], BF16,
                              kind="ExternalOutput").ap()
        dbgf = nc.dram_tensor("dbgf", [66, 512], F32,
                              kind="ExternalOutput").ap()

    with tile.TileContext(nc) as tc, ExitStack() as ctx:
        sb = ctx.enter_context(tc.tile_pool(name="sb", bufs=1))
        ps = ctx.enter_context(tc.tile_pool(name="ps", bufs=1, space="PSUM"))

        x_sb = [sb.tile([128, 2048], BF16, name=f"x{j}") for j in range(8)]
        wq_sb = [sb.tile([128, 512], BF16, name=f"wq{j}") for j in range(8)]
        wk_sb = [sb.tile([128, 512], BF16, name=f"wk{j}") for j in range(8)]
        wv_sb = [sb.tile([128, 512], BF16, name=f"wv{j}") for j in range(8)]
        pw_sb = sb.tile([128, 4, 1024], BF16, tag="pw")
        QT = [sb.tile([128, 2048], BF16, name=f"QT{p}") for p in range(4)]
        KT = [sb.tile([128, 2048], BF16, name=f"KT{p}") for p in range(4)]
        VT = [sb.tile([128, 16, 2, 65], BF16, name=f"VT{p}") for p in range(4)]
        OT = [sb.tile([128, 2048], BF16, name=f"OT{p}") for p in range(4)]
        qb_sb = sb.tile([128, 4], F32, tag="qb")
        kb_sb = sb.tile([128, 4], F32, tag="kb")
        zc = sb.tile([128, 16, 2, 1], F32, tag="zc")
        onec = sb.tile([128, 1], F32, tag="onec")

        # loads ordered by first use: Q weights + x stream, then K, V, proj.
        # x goes out on the scalar engine's DMA queue, weights on sync's, so
        # the two streams transfer in parallel and the first QKV matmul can
        # start ~8us earlier.
        for j in range(8):
            nc.sync.dma_start(wq_sb[j][:], wcat[j * 128:(j + 1) * 128, 0:512])
            nc.sync.dma_start(x_sb[j][:], xT[j * 128:(j + 1) * 128, :])
        nc.sync.dma_start(qb_sb[:], qbias[:])
        nc.sync.dma_start(kb_sb[:], kbias[:])
        for j in range(8):
            nc.sync.dma_start(wk_sb[j][:], wcat[j * 128:(j + 1) * 128, 512:1024])
        for j in range(8):
            nc.sync.dma_start(wv_sb[j][:], wcat[j * 128:(j + 1) * 128, 1024:1536])
        nc.vector.memset(zc[:], 0.0)
        nc.vector.memset(onec[:], 1.0)
        for pr in range(4):
            nc.sync.dma_start(pw_sb[:, pr, :], pw[pr * 128:(pr + 1) * 128, :])
            nc.vector.tensor_scalar(out=VT[pr][:, :, :, 64:65], in0=zc[:],
                                    scalar1=onec[:], scalar2=None, op0=ADD)

        def qkv_groups(pr):
            """32 emitters: (owner_pr, pe_cycles, fn).  K comes first (the
            next head-pair's first scores need the full K panel the moment
            its attention starts), then Q for query-block 0, then V, then the
            remaining Q blocks (needed one query-block at a time)."""
            def gq(nb):
                acc = ps.tile([128, 512], F32, tag="acc", bufs=2)
                for j in range(8):
                    nc.tensor.matmul(acc[:, 0:256],
                                     wq_sb[j][:, pr * 128:(pr + 1) * 128],
                                     x_sb[j][:, nb * 256:(nb + 1) * 256],
                                     start=(j == 0), stop=(j == 7))
                nc.vector.tensor_scalar(
                    out=QT[pr][:, nb * 256:(nb + 1) * 256], in0=acc[:, 0:256],
                    scalar1=qb_sb[:, pr:pr + 1], scalar2=None, op0=ADD)

            def gk(nb):
                acc = ps.tile([128, 512], F32, tag="acc", bufs=2)
                for j in range(8):
                    nc.tensor.matmul(acc[:, 0:256],
                                     wk_sb[j][:, pr * 128:(pr + 1) * 128],
                                     x_sb[j][:, nb * 256:(nb + 1) * 256],
                                     start=(j == 0), stop=(j == 7))
                nc.vector.tensor_scalar(
                    out=KT[pr][:, nb * 256:(nb + 1) * 256], in0=acc[:, 0:256],
                    scalar1=kb_sb[:, pr:pr + 1], scalar2=None, op0=ADD)

            def gv(t):
                acc = ps.tile([128, 512], F32, tag="acc", bufs=2)
                for j in range(8):
                    nc.tensor.matmul(acc[:, 0:128],
                                     x_sb[j][:, t * 128:(t + 1) * 128],
                                     wv_sb[j][:, pr * 128:(pr + 1) * 128],
                                     start=(j == 0), stop=(j == 7))
                nc.vector.tensor_copy(
                    out=VT[pr][:, t, :, 0:64],
                    in_=acc[:, 0:128].rearrange("p (h d) -> p h d", h=2))

            import functools
            for nb in range(8):
                yield pr, 2048, functools.partial(gk, nb)
            for nb in range(2):
                yield pr, 2048, functools.partial(gq, nb)
            for t in range(16):
                yield pr, 1024, functools.partial(gv, t)
            for nb in range(2, 8):
                yield pr, 2048, functools.partial(gq, nb)

        def proj_groups(qb):
            """8 emitters: output projection for query block qb."""
            q0 = qb * 512
            for ns in range(4):
                for co in range(2):
                    def gp(ns=ns, co=co):
                        pj = ps.tile([128, 512], F32, tag="acc", bufs=2)
                        for pr4 in range(4):
                            nc.tensor.matmul(
                                pj[:],
                                OT[pr4][:, q0 + ns * 128:q0 + (ns + 1) * 128],
                                pw_sb[:, pr4, co * 512:(co + 1) * 512],
                                start=(pr4 == 0), stop=(pr4 == 3))
                        so = sb.tile([128, 512], F32, tag="so", bufs=2)
                        nc.vector.tensor_copy(out=so[:], in_=pj[:])
                        nc.sync.dma_start(
                            out[q0 + ns * 128:q0 + (ns + 1) * 128,
                                co * 512:(co + 1) * 512], so[:])
                    yield 9, 2048, gp

        # Global fill queue: QKV for later head-pairs and the output
        # projection are paced evenly into the attention tg slots (~1.4k PE
        # cycles per slot) so the tensor engine always stays ahead of the
        # scalar engine's exp stream.
        fills = deque()
        pending = deque()  # deferred normalize tails (DVE divide)
        st8 = {"filled": 0, "target": 0}

        def pace():
            st8["target"] += 1365
            popped = 0
            while fills and st8["filled"] < st8["target"] and popped < 2:
                _, cyc, g = fills.popleft()
                g()
                st8["filled"] += cyc
                popped += 1

        def flush(owner_max):
            while fills and fills[0][0] <= owner_max:
                _, cyc, g = fills.popleft()
                g()
                st8["filled"] += cyc

        def attn(pr, qb, late=None):
            q0 = qb * 512
            oaug0 = ps.tile([65, 512], F32, tag="oaug", bufs=2)
            oaug1 = ps.tile([65, 512], F32, tag="oaug", bufs=2)
            staged = []
            for tg in range(9):
                if tg < 8:
                    t0, t1 = 2 * tg, 2 * tg + 1
                    stage0 = ps.tile([128, 1024], F32, tag="stage", bufs=2)
                    stage1 = ps.tile([128, 1024], F32, tag="stage", bufs=2)
                    # scores S^T [keys, queries]; heads (2pr, 2pr+1) row-packed
                    nc.tensor.matmul(stage0[:, 0:512],
                                     KT[pr][0:64, t0 * 128:(t0 + 1) * 128],
                                     QT[pr][0:64, q0:q0 + 512],
                                     start=True, stop=True, tile_position=(0, 0))
                    nc.tensor.matmul(stage1[:, 0:512],
                                     KT[pr][64:128, t0 * 128:(t0 + 1) * 128],
                                     QT[pr][64:128, q0:q0 + 512],
                                     start=True, stop=True, tile_position=(64, 0))
                    nc.tensor.matmul(stage0[:, 512:1024],
                                     KT[pr][0:64, t1 * 128:(t1 + 1) * 128],
                                     QT[pr][0:64, q0:q0 + 512],
                                     start=True, stop=True, tile_position=(0, 0))
                    nc.tensor.matmul(stage1[:, 512:1024],
                                     KT[pr][64:128, t1 * 128:(t1 + 1) * 128],
                                     QT[pr][64:128, q0:q0 + 512],
                                     start=True, stop=True, tile_position=(64, 0))
                if tg >= 1:
                    # PV lags S by one tg so exp overlaps the next S pair
                    pP0, pP1, pt0, pt1 = staged[tg - 1]
                    st, sp = (tg - 1 == 0), (tg - 1 == 7)
                    nc.tensor.matmul(oaug0[:], VT[pr][:, pt0, 0, :],
                                     pP0[:, 0:512], start=st, stop=False)
                    nc.tensor.matmul(oaug0[:], VT[pr][:, pt1, 0, :],
                                     pP0[:, 512:1024], start=False, stop=sp)
                    nc.tensor.matmul(oaug1[:], VT[pr][:, pt0, 1, :],
                                     pP1[:, 0:512], start=st, stop=False)
                    nc.tensor.matmul(oaug1[:], VT[pr][:, pt1, 1, :],
                                     pP1[:, 512:1024], start=False, stop=sp)
                if tg < 8:
                    P0 = sb.tile([128, 1024], BF16, tag="p", bufs=4)
                    P1 = sb.tile([128, 1024], BF16, tag="p", bufs=4)
                    nc.scalar.activation(P0[:], stage0[:], EXP,
                                         bias=0.0, scale=SCALE)
                    nc.scalar.activation(P1[:], stage1[:], EXP,
                                         bias=0.0, scale=SCALE)
                    if dbg and pr == 0 and qb == 0 and tg == 0:
                        nc.sync.dma_start(dbgb[4:5, 0:1024], P0[0:1, :])
                        nc.sync.dma_start(dbgb[5:6, 0:1024], P1[0:1, :])
                    staged.append((P0, P1, t0, t1))
                if tg == 7 and late is not None:
                    fills.extend(late)
                pace()
                if pending:
                    pending.popleft()()
            # normalization, off the critical path: free oaug via SBUF copy,
            # approx-reciprocal the denominator row, DRAM-bounce broadcast,
            # scale into OT.
            # Normalize tail, deferred into the next attention block: oaug is
            # freed via an immediate DVE copy; the serial DVE reciprocal is
            # split into [1,256] halves dripped one per tg slot so the DVE
            # never builds a backlog ahead of the PSUM-recycling ops that gate
            # the tensor engine; the final row-broadcast multiply runs on the
            # otherwise-idle GPSIMD/Pool engine.
            osts, rcs, rbs = [], [], []
            for hh, oaug in ((0, oaug0), (1, oaug1)):
                ost = sb.tile([65, 512], F32, tag="ost", bufs=4)
                nc.vector.tensor_copy(out=ost[:], in_=oaug[:])
                if dbg and pr == 0 and qb == 0 and hh == 0:
                    nc.sync.dma_start(dbgf[0:65, :], ost[:])
                osts.append(ost)
                rcs.append(sb.tile([1, 512], F32, tag="rc", bufs=4, name="rc"))
                rbs.append(sb.tile([64, 512], F32, tag="rb", bufs=4, name="rb"))

            final = pr == 3 and qb == 3

            def recip_piece(hh, half, pr=pr, qb=qb, osts=osts, rcs=rcs):
                row = pr * 8 + qb * 2 + hh
                lo, hi = half * 256, (half + 1) * 256
                nc.vector.reciprocal(rcs[hh][0:1, lo:hi], osts[hh][64:65, lo:hi])
                nc.sync.dma_start(scratch[row:row + 1, lo:hi],
                                  rcs[hh][0:1, lo:hi])

            def recip_act(hh, pr=pr, qb=qb, osts=osts, rcs=rcs):
                # final query block: all exps are done, so the scalar engine
                # is free — 1/d = exp(-ln d) via the ln+exp activation table
                # keeps the DVE clear for the projection's PSUM recycling.
                row = pr * 8 + qb * 2 + hh
                lns = sb.tile([1, 512], F32, tag="lns", bufs=2, name="lns")
                nc.scalar.activation(lns[:], osts[hh][64:65, :],
                                     mybir.ActivationFunctionType.Ln,
                                     bias=0.0, scale=1.0)
                nc.scalar.activation(rcs[hh][0:1, :], lns[:], EXP,
                                     bias=0.0, scale=-1.0)
                nc.sync.dma_start(scratch[row:row + 1, :], rcs[hh][0:1, :])

            def bcast_piece(pr=pr, qb=qb, rbs=rbs):
                for hh in range(2):
                    row = pr * 8 + qb * 2 + hh
                    nc.sync.dma_start(
                        rbs[hh][:],
                        scratch[row:row + 1, :].to_broadcast((64, 512)))

            def norm_piece(hh, pr=pr, q0=q0, osts=osts, rbs=rbs):
                nc.gpsimd.tensor_tensor(out=OT[pr][hh * 64:(hh + 1) * 64,
                                                   q0:q0 + 512],
                                        in0=osts[hh][0:64, :], in1=rbs[hh][:],
                                        op=MULT)

            if final:
                recip_act(0)
                recip_act(1)
                bcast_piece()
                norm_piece(0)
                norm_piece(1)
            else:
                pending.extend([
                    lambda: recip_piece(0, 0), lambda: recip_piece(1, 0),
                    lambda: recip_piece(0, 1), lambda: recip_piece(1, 1),
                    bcast_piece,
                    lambda: norm_piece(0), lambda: norm_piece(1),
                ])

        for _, _, g in qkv_groups(0):
            g()
        if dbg:
            nc.sync.dma_start(dbgb[0:1, :], QT[0][0:1, :])
            nc.sync.dma_start(dbgb[1:2, :], KT[0][0:1, :])
            nc.sync.dma_start(dbgb[2:3, 0:65], VT[0][0:1, 0, 0, :])
            nc.sync.dma_start(dbgb[3:4, 0:65], VT[0][0:1, 0, 1, :])
        for p in (1, 2, 3):
            fills.extend(qkv_groups(p))
        for pr in range(4):
            flush(pr)  # QKV(pr) must be fully emitted before its attention
            for qb in range(4):
                late = proj_groups(qb - 1) if pr == 3 and qb >= 1 else None
                attn(pr, qb, late=late)
        while pending:
            pending.popleft()()
        while fills:
            fills.popleft()[2]()
        for _, _, g in proj_groups(3):
            g()
    return nc


def _prepare_in_maps(x, qkv_w, qkv_b, proj_w):
    x = np.asarray(x, dtype=np.float32)
    wb = np.asarray(qkv_w, dtype=np.float32).astype(bfloat16)
    pwb = np.asarray(proj_w, dtype=np.float32).astype(bfloat16)
    qkv_b = np.asarray(qkv_b, dtype=np.float32)
    in_maps = []
    for c in range(8):
        b, g = c % 4, c // 4
        w0 = 512 * g
        in_maps.append({
            "xT": np.ascontiguousarray(x[b].T).astype(bfloat16),
            "wcat": np.ascontiguousarray(np.concatenate(
                [wb[:, w0:w0 + 512],
                 wb[:, 1024 + w0:1024 + w0 + 512],
                 wb[:, 2048 + w0:2048 + w0 + 512]], axis=1)),
            "qb": np.ascontiguousarray(qkv_b[w0:w0 + 512].reshape(4, 128).T),
            "kb": np.ascontiguousarray(
                qkv_b[1024 + w0:1024 + w0 + 512].reshape(4, 128).T),
            "pw": np.ascontiguousarray(pwb[w0:w0 + 512, :]),
        })
    return in_maps


def _gather(parts, qkv_b, proj_w, proj_b):
    const_row = (np.asarray(qkv_b)[2048:].astype(np.float64)
                 @ np.asarray(proj_w).astype(np.float64)
                 + np.asarray(proj_b).astype(np.float64))
    out = np.empty((B, N, C), np.float32)
    for b in range(B):
        out[b] = (parts[b].astype(np.float64) + parts[b + 4].astype(np.float64)
                  + const_row).astype(np.float32)
    return out


def kernel(**inputs: np.ndarray) -> np.ndarray:
    x = np.asarray(inputs["x"], dtype=np.float32)
    qkv_w = np.asarray(inputs["qkv_w"], dtype=np.float32)
    qkv_b = np.asarray(inputs["qkv_b"], dtype=np.float32)
    proj_w = np.asarray(inputs["proj_w"], dtype=np.float32)
    proj_b = np.asarray(inputs["proj_b"], dtype=np.float32)

    in_maps = _prepare_in_maps(x, qkv_w, qkv_b, proj_w)
    nc = _build()
    nc.finalize()
    res = run_bass_kernel_spmd(nc, in_maps, list(range(8)))
    parts = [res.results[c]["out"] for c in range(8)]
    return _gather(parts, qkv_b, proj_w, proj_b)


if __name__ == "__main__":
    import tempfile
    import time

    from concourse.bass_utils import compile_bass_kernel

    t0 = time.time()
    nc = _build()
    nc.compile()
    with tempfile.TemporaryDirectory() as td:
        compile_bass_kernel(nc, td, neff_name="k.neff")
    print(f"COMPILE OK ({time.time() - t0:.0f}s)", flush=True)


# revision 45
# speedup vs baseline: 1.2723x; 1.0013x over previous
"""Multi-head attention (B=4, N=2048, C=1024, H=16, D=64) on 8 TRN2 cores.

Sharding: core c -> batch b = c%4, head-group g = c//4 (local heads 0..7 are
global heads 8g..8g+7).  Each core computes its head group's contribution to
the output projection for its batch; host sums core b + core b+4 and adds
const_row = qkv_b[2048:] @ proj_w + proj_b (V-bias folds exactly through the
row-normalized attention: attn @ (1*bv^T) = 1*bv^T).

v2: all-bf16 datapath (inputs pre-cast on host), software-pipelined schedule:
QKV projection for head-pair pr+1 is interleaved into the attention tg-loop of
head-pair pr so the scalar engine's exp stream (the phase-2 co-bottleneck)
overlaps the tensor engine's QKV matmuls instead of idling during a separate
phase 1.  Projection for query block qb is interleaved into attention of
(pr=3, qb+1).  Softmax normalization moved off the critical path: oaug PSUM is
copied to SBUF immediately (freeing the PSUM bank for the next query block),
then reciprocal_approx_fast + DRAM-bounce broadcast + multiply trail behind.

Per-core device layout:
  x_sb[j]  [128, 2048] bf16   xT rows j*128.. (C x N), j = 0..7
  wq/wk/wv_sb[j] [128, 512]   W columns for this head group, per input-row j
  QT/KT[pr] [128, 2048] bf16  partition = dim-in-pair (2 heads x 64), free = tok
  VT[pr]   [128, 16, 2, 65]   partition = token-in-block; col 64 = ones (denom)
  OT[pr]   [128, 2048] bf16   normalized attention output, chan-pair x token
  out      [2048, 1024] f32   partial projection output
"""

import sys

sys.path.insert(0, "/opt/trn_rl_repo")

from collections import deque
from contextlib import ExitStack

import numpy as np
from ml_dtypes import bfloat16

from concourse import bacc, mybir, tile
from concourse.bass_utils import run_bass_kernel_spmd

F32 = mybir.dt.float32
BF16 = mybir.dt.bfloat16
EXP = mybir.ActivationFunctionType.Exp
ADD = mybir.AluOpType.add
MULT = mybir.AluOpType.mult
DIV = mybir.AluOpType.divide

B, N, C, H, D = 4, 2048, 1024, 16, 64
SCALE = 0.125


def _build(dbg=False):
    nc = bacc.Bacc("TRN2", target_bir_lowering=False, debug=False)
    xT = nc.dram_tensor("xT", [1024, 2048], BF16, kind="ExternalInput").ap()
    wcat = nc.dram_tensor("wcat", [1024, 1536], BF16, kind="ExternalInput").ap()
    qbias = nc.dram_tensor("qb", [128, 4], F32, kind="ExternalInput").ap()
    kbias = nc.dram_tensor("kb", [128, 4], F32, kind="ExternalInput").ap()
    pw = nc.dram_tensor("pw", [512, 1024], BF16, kind="ExternalInput").ap()
    out = nc.dram_tensor("out", [2048, 1024], BF16, kind="ExternalOutput").ap()
    scratch = nc.dram_tensor("scratch", [32, 512], F32).ap()
    if dbg:
        dbgb = nc.dram_tensor("dbgb", [8, 2048], BF16,
                              kind="ExternalOutput").ap()
        dbgf = nc.dram_tensor("dbgf", [66, 512], F32,
                              kind="ExternalOutput").ap()

    with tile.TileContext(nc) as tc, ExitStack() as ctx:
        sb = ctx.enter_context(tc.tile_pool(name="sb", bufs=1))
        ps = ctx.enter_context(tc.tile_pool(name="ps", bufs=1, space="PSUM"))

        x_sb = [sb.tile([128, 2048], BF16, name=f"x{j}") for j in range(8)]
        wq_sb = [sb.tile([128, 512], BF16, name=f"wq{j}") for j in range(8)]
        wk_sb = [sb.tile([128, 512], BF16, name=f"wk{j}") for j in range(8)]
        wv_sb = [sb.tile([128, 512], BF16, name=f"wv{j}") for j in range(8)]
        pw_sb = sb.tile([128, 4, 1024], BF16, tag="pw")
        QT = [sb.tile([128, 2048], BF16, name=f"QT{p}") for p in range(4)]
        KT = [sb.tile([128, 2048], BF16, name=f"KT{p}") for p in range(4)]
        VT = [sb.tile([128, 16, 2, 65], BF16, name=f"VT{p}") for p in range(4)]
        OT = [sb.tile([128, 2048], BF16, name=f"OT{p}") for p in range(4)]
        qb_sb = sb.tile([128, 4], F32, tag="qb")
        kb_sb = sb.tile([128, 4], F32, tag="kb")
        zc = sb.tile([128, 16, 2, 1], F32, tag="zc")
        onec = sb.tile([128, 1], F32, tag="onec")

        # loads ordered by first use: Q weights + x stream, then K, V, proj.
        # x goes out on the scalar engine's DMA queue, weights on sync's, so
        # the two streams transfer in parallel and the first QKV matmul can
        # start ~8us earlier.
        for j in range(8):
            nc.sync.dma_start(wq_sb[j][:], wcat[j * 128:(j + 1) * 128, 0:512])
            nc.sync.dma_start(x_sb[j][:], xT[j * 128:(j + 1) * 128, :])
        nc.sync.dma_start(qb_sb[:], qbias[:])
        nc.sync.dma_start(kb_sb[:], kbias[:])
        for j in range(8):
            nc.sync.dma_start(wk_sb[j][:], wcat[j * 128:(j + 1) * 128, 512:1024])
        for j in range(8):
            nc.sync.dma_start(wv_sb[j][:], wcat[j * 128:(j + 1) * 128, 1024:1536])
        nc.vector.memset(zc[:], 0.0)
        nc.vector.memset(onec[:], 1.0)
        for pr in range(4):
            nc.sync.dma_start(pw_sb[:, pr, :], pw[pr * 128:(pr + 1) * 128, :])
            nc.vector.tensor_scalar(out=VT[pr][:, :, :, 64:65], in0=zc[:],
                                    scalar1=onec[:], scalar2=None, op0=ADD)

        def qkv_groups(pr):
            """32 emitters: (owner_pr, pe_cycles, fn).  K comes first (the
            next head-pair's first scores need the full K panel the moment
            its attention starts), then Q for query-block 0, then V, then the
            remaining Q blocks (needed one query-block at a time)."""
            def gq(nb):
                acc = ps.tile([128, 512], F32, tag="acc", bufs=2)
                for j in range(8):
                    nc.tensor.matmul(acc[:, 0:256],
                                     wq_sb[j][:, pr * 128:(pr + 1) * 128],
                                     x_sb[j][:, nb * 256:(nb + 1) * 256],
                                     start=(j == 0), stop=(j == 7))
                nc.vector.tensor_scalar(
                    out=QT[pr][:, nb * 256:(nb + 1) * 256], in0=acc[:, 0:256],
                    scalar1=qb_sb[:, pr:pr + 1], scalar2=None, op0=ADD)

            def gk(nb):
                acc = ps.tile([128, 512], F32, tag="acc", bufs=2)
                for j in range(8):
                    nc.tensor.matmul(acc[:, 0:256],
                                     wk_sb[j][:, pr * 128:(pr + 1) * 128],
                                     x_sb[j][:, nb * 256:(nb + 1) * 256],
                                     start=(j == 0), stop=(j == 7))
                nc.vector.tensor_scalar(
                    out=KT[pr][:, nb * 256:(nb + 1) * 256], in0=acc[:, 0:256],
                    scalar1=kb_sb[:, pr:pr + 1], scalar2=None, op0=ADD)

            def gv(t):
                acc = ps.tile([128, 512], F32, tag="acc", bufs=2)
                for j in range(8):
                    nc.tensor.matmul(acc[:, 0:128],
                                     x_sb[j][:, t * 128:(t + 1) * 128],
                                     wv_sb[j][:, pr * 128:(pr + 1) * 128],
                                     start=(j == 0), stop=(j == 7))
                nc.vector.tensor_copy(
                    out=VT[pr][:, t, :, 0:64],
                    in_=acc[:, 0:128].rearrange("p (h d) -> p h d", h=2))

            import functools
            if pr == 0:
                # upfront batch: follow DMA arrival order (wq+x first)
                for nb in range(8):
                    yield pr, 2048, functools.partial(gq, nb)
                for nb in range(8):
                    yield pr, 2048, functools.partial(gk, nb)
                for t in range(16):
                    yield pr, 1024, functools.partial(gv, t)
            else:
                for nb in range(8):
                    yield pr, 2048, functools.partial(gk, nb)
                for nb in range(2):
                    yield pr, 2048, functools.partial(gq, nb)
                for t in range(16):
                    yield pr, 1024, functools.partial(gv, t)
                for nb in range(2, 8):
                    yield pr, 2048, functools.partial(gq, nb)

        def proj_groups(qb, tail=False):
            """8 emitters: output projection for query block qb.  In the tail
            (after the last exp) the scalar engine is idle: its Copy drains
            PSUM and both the acc and oaug rings are free, halving ring-
            recycle stalls; output DMAs split across two engine queues."""
            q0 = qb * 512
            for gi, (ns, co) in enumerate((n, c) for n in range(4)
                                          for c in range(2)):
                def gp(ns=ns, co=co, gi=gi):
                    ring = ("oaug" if (tail and gi % 2) else "acc")
                    pj = ps.tile([128, 512], F32, tag=ring, bufs=2, name="pj")
                    for pr4 in range(4):
                        nc.tensor.matmul(
                            pj[:],
                            OT[pr4][:, q0 + ns * 128:q0 + (ns + 1) * 128],
                            pw_sb[:, pr4, co * 512:(co + 1) * 512],
                            start=(pr4 == 0), stop=(pr4 == 3))
                    so = sb.tile([128, 512], BF16, tag="so", bufs=4, name="so")
                    if tail:
                        nc.scalar.activation(so[:], pj[:],
                                             mybir.ActivationFunctionType.Copy,
                                             bias=0.0, scale=1.0)
                    else:
                        nc.vector.tensor_copy(out=so[:], in_=pj[:])
                    eng = nc.scalar if (tail and gi % 2) else nc.sync
                    eng.dma_start(
                        out[q0 + ns * 128:q0 + (ns + 1) * 128,
                            co * 512:(co + 1) * 512], so[:])
                yield 9, 2048, gp

        # Global fill queue: QKV for later head-pairs and the output
        # projection are paced evenly into the attention tg slots (~1.4k PE
        # cycles per slot) so the tensor engine always stays ahead of the
        # scalar engine's exp stream.
        fills = deque()
        pending = deque()  # deferred normalize tails (DVE divide)
        st8 = {"filled": 0, "target": 0}

        def pace():
            st8["target"] += 1550
            popped = 0
            while fills and st8["filled"] < st8["target"] and popped < 2:
                _, cyc, g = fills.popleft()
                g()
                st8["filled"] += cyc
                popped += 1

        def flush(owner_max):
            while fills and fills[0][0] <= owner_max:
                _, cyc, g = fills.popleft()
                g()
                st8["filled"] += cyc

        def attn(pr, qb, late=None):
            q0 = qb * 512
            oaug0 = ps.tile([65, 512], F32, tag="oaug", bufs=2)
            oaug1 = ps.tile([65, 512], F32, tag="oaug", bufs=2)
            staged = []
            for tg in range(9):
                if tg < 8:
                    t0, t1 = 2 * tg, 2 * tg + 1
                    stage0 = ps.tile([128, 1024], F32, tag="stage", bufs=2)
                    stage1 = ps.tile([128, 1024], F32, tag="stage", bufs=2)
                    # scores S^T [keys, queries]; heads (2pr, 2pr+1) row-packed
                    nc.tensor.matmul(stage0[:, 0:512],
                                     KT[pr][0:64, t0 * 128:(t0 + 1) * 128],
                                     QT[pr][0:64, q0:q0 + 512],
                                     start=True, stop=True, tile_position=(0, 0))
                    nc.tensor.matmul(stage1[:, 0:512],
                                     KT[pr][64:128, t0 * 128:(t0 + 1) * 128],
                                     QT[pr][64:128, q0:q0 + 512],
                                     start=True, stop=True, tile_position=(64, 0))
                    nc.tensor.matmul(stage0[:, 512:1024],
                                     KT[pr][0:64, t1 * 128:(t1 + 1) * 128],
                                     QT[pr][0:64, q0:q0 + 512],
                                     start=True, stop=True, tile_position=(0, 0))
                    nc.tensor.matmul(stage1[:, 512:1024],
                                     KT[pr][64:128, t1 * 128:(t1 + 1) * 128],
                                     QT[pr][64:128, q0:q0 + 512],
                                     start=True, stop=True, tile_position=(64, 0))
                if tg >= 1:
                    # PV lags S by one tg so exp overlaps the next S pair
                    pP0, pP1, pt0, pt1 = staged[tg - 1]
                    st, sp = (tg - 1 == 0), (tg - 1 == 7)
                    nc.tensor.matmul(oaug0[:], VT[pr][:, pt0, 0, :],
                                     pP0[:, 0:512], start=st, stop=False)
                    nc.tensor.matmul(oaug0[:], VT[pr][:, pt1, 0, :],
                                     pP0[:, 512:1024], start=False, stop=sp)
                    nc.tensor.matmul(oaug1[:], VT[pr][:, pt0, 1, :],
                                     pP1[:, 0:512], start=st, stop=False)
                    nc.tensor.matmul(oaug1[:], VT[pr][:, pt1, 1, :],
                                     pP1[:, 512:1024], start=False, stop=sp)
                if tg < 8:
                    P0 = sb.tile([128, 1024], BF16, tag="p", bufs=4)
                    P1 = sb.tile([128, 1024], BF16, tag="p", bufs=4)
                    nc.scalar.activation(P0[:], stage0[:], EXP,
                                         bias=0.0, scale=SCALE)
                    nc.scalar.activation(P1[:], stage1[:], EXP,
                                         bias=0.0, scale=SCALE)
                    if dbg and pr == 0 and qb == 0 and tg == 0:
                        nc.sync.dma_start(dbgb[4:5, 0:1024], P0[0:1, :])
                        nc.sync.dma_start(dbgb[5:6, 0:1024], P1[0:1, :])
                    staged.append((P0, P1, t0, t1))
                if tg == 7 and late is not None:
                    fills.extend(late)
                pace()
                if pending:
                    pending.popleft()()
            # normalization, off the critical path: free oaug via SBUF copy,
            # approx-reciprocal the denominator row, DRAM-bounce broadcast,
            # scale into OT.
            # Normalize tail, deferred into the next attention block: oaug is
            # freed via an immediate DVE copy; the serial DVE reciprocal is
            # split into [1,256] halves dripped one per tg slot so the DVE
            # never builds a backlog ahead of the PSUM-recycling ops that gate
            # the tensor engine; the final row-broadcast multiply runs on the
            # otherwise-idle GPSIMD/Pool engine.
            osts, rcs, rbs = [], [], []
            for hh, oaug in ((0, oaug0), (1, oaug1)):
                ost = sb.tile([65, 512], F32, tag="ost", bufs=4)
                nc.vector.tensor_copy(out=ost[:], in_=oaug[:])
                if dbg and pr == 0 and qb == 0 and hh == 0:
                    nc.sync.dma_start(dbgf[0:65, :], ost[:])
                osts.append(ost)
                rcs.append(sb.tile([1, 512], F32, tag="rc", bufs=4, name="rc"))
                rbs.append(sb.tile([64, 512], F32, tag="rb", bufs=4, name="rb"))

            final = pr == 3 and qb == 3

            def recip_piece(hh, half, pr=pr, qb=qb, osts=osts, rcs=rcs):
                row = pr * 8 + qb * 2 + hh
                lo, hi = half * 256, (half + 1) * 256
                nc.vector.reciprocal(rcs[hh][0:1, lo:hi], osts[hh][64:65, lo:hi])
                nc.sync.dma_start(scratch[row:row + 1, lo:hi],
                                  rcs[hh][0:1, lo:hi])

            def recip_act(hh, pr=pr, qb=qb, osts=osts, rcs=rcs):
                # final query block: all exps are done, so the scalar engine
                # is free — 1/d = exp(-ln d) via the ln+exp activation table
                # keeps the DVE clear for the projection's PSUM recycling.
                row = pr * 8 + qb * 2 + hh
                lns = sb.tile([1, 512], F32, tag="lns", bufs=2, name="lns")
                nc.scalar.activation(lns[:], osts[hh][64:65, :],
                                     mybir.ActivationFunctionType.Ln,
                                     bias=0.0, scale=1.0)
                nc.scalar.activation(rcs[hh][0:1, :], lns[:], EXP,
                                     bias=0.0, scale=-1.0)
                nc.sync.dma_start(scratch[row:row + 1, :], rcs[hh][0:1, :])

            def bcast_piece(pr=pr, qb=qb, rbs=rbs):
                for hh in range(2):
                    row = pr * 8 + qb * 2 + hh
                    nc.sync.dma_start(
                        rbs[hh][:],
                        scratch[row:row + 1, :].to_broadcast((64, 512)))

            def norm_piece(hh, pr=pr, q0=q0, osts=osts, rbs=rbs):
                nc.gpsimd.tensor_tensor(out=OT[pr][hh * 64:(hh + 1) * 64,
                                                   q0:q0 + 512],
                                        in0=osts[hh][0:64, :], in1=rbs[hh][:],
                                        op=MULT)

            if final:
                recip_act(0)
                recip_act(1)
                bcast_piece()
                norm_piece(0)
                norm_piece(1)
            else:
                pending.extend([
                    lambda: recip_piece(0, 0), lambda: recip_piece(1, 0),
                    lambda: recip_piece(0, 1), lambda: recip_piece(1, 1),
                    bcast_piece,
                    lambda: norm_piece(0), lambda: norm_piece(1),
                ])

        for _, _, g in qkv_groups(0):
            g()
        if dbg:
            nc.sync.dma_start(dbgb[0:1, :], QT[0][0:1, :])
            nc.sync.dma_start(dbgb[1:2, :], KT[0][0:1, :])
            nc.sync.dma_start(dbgb[2:3, 0:65], VT[0][0:1, 0, 0, :])
            nc.sync.dma_start(dbgb[3:4, 0:65], VT[0][0:1, 0, 1, :])
        for p in (1, 2, 3):
            fills.extend(qkv_groups(p))
        for pr in range(4):
            flush(pr)  # QKV(pr) must be fully emitted before its attention
            for qb in range(4):
                late = proj_groups(qb - 1) if pr == 3 and qb >= 1 else None
                attn(pr, qb, late=late)
        while pending:
            pending.popleft()()
        while fills:
            fills.popleft()[2]()
        for _, _, g in proj_groups(3, tail=True):
            g()
    return nc


def _prepare_in_maps(x, qkv_w, qkv_b, proj_w):
    x = np.asarray(x, dtype=np.float32)
    wb = np.asarray(qkv_w, dtype=np.float32).astype(bfloat16)
    pwb = np.asarray(proj_w, dtype=np.float32).astype(bfloat16)
    qkv_b = np.asarray(qkv_b, dtype=np.float32)
    in_maps = []
    for c in range(8):
        b, g = c % 4, c // 4
        w0 = 512 * g
        in_maps.append({
            "xT": np.ascontiguousarray(x[b].T).astype(bfloat16),
            "wcat": np.ascontiguousarray(np.concatenate(
                [wb[:, w0:w0 + 512],
                 wb[:, 1024 + w0:1024 + w0 + 512],
                 wb[:, 2048 + w0:2048 + w0 + 512]], axis=1)),
            "qb": np.ascontiguousarray(qkv_b[w0:w0 + 512].reshape(4, 128).T),
            "kb": np.ascontiguousarray(
                qkv_b[1024 + w0:1024 + w0 + 512].reshape(4, 128).T),
            "pw": np.ascontiguousarray(pwb[w0:w0 + 512, :]),
        })
    return in_maps


def _gather(parts, qkv_b, proj_w, proj_b):
    const_row = (np.asarray(qkv_b)[2048:].astype(np.float64)
                 @ np.asarray(proj_w).astype(np.float64)
                 + np.asarray(proj_b).astype(np.float64))
    out = np.empty((B, N, C), np.float32)
    for b in range(B):
        out[b] = (parts[b].astype(np.float64) + parts[b + 4].astype(np.float64)
                  + const_row).astype(np.float32)
    return out


def kernel(**inputs: np.ndarray) -> np.ndarray:
    x = np.asarray(inputs["x"], dtype=np.float32)
    qkv_w = np.asarray(inputs["qkv_w"], dtype=np.float32)
    qkv_b = np.asarray(inputs["qkv_b"], dtype=np.float32)
    proj_w = np.asarray(inputs["proj_w"], dtype=np.float32)
    proj_b = np.asarray(inputs["proj_b"], dtype=np.float32)

    in_maps = _prepare_in_maps(x, qkv_w, qkv_b, proj_w)
    nc = _build()
    nc.finalize()
    res = run_bass_kernel_spmd(nc, in_maps, list(range(8)))
    parts = [res.results[c]["out"] for c in range(8)]
    return _gather(parts, qkv_b, proj_w, proj_b)


if __name__ == "__main__":
    import tempfile
    import time

    from concourse.bass_utils import compile_bass_kernel

    t0 = time.time()
    nc = _build()
    nc.compile()
    with tempfile.TemporaryDirectory() as td:
        compile_bass_kernel(nc, td, neff_name="k.neff")
    print(f"COMPILE OK ({time.time() - t0:.0f}s)", flush=True)


# revision 46
# speedup vs baseline: 1.2968x; 1.0193x over previous
"""Multi-head attention (B=4, N=2048, C=1024, H=16, D=64) on 8 TRN2 cores.

Sharding: core c -> batch b = c%4, head-group g = c//4 (local heads 0..7 are
global heads 8g..8g+7).  Each core computes its head group's contribution to
the output projection for its batch; host sums core b + core b+4 and adds
const_row = qkv_b[2048:] @ proj_w + proj_b (V-bias folds exactly through the
row-normalized attention: attn @ (1*bv^T) = 1*bv^T).

v3: all-bf16 datapath; one seamless software pipeline over every
(head-pair pr, query-block qb, key-block-pair tg) slot so the scores ->
exp(scalar engine) stream NEVER pauses at qb/pr boundaries (the exp stream is
the co-bottleneck: ~285us of scalar-engine work vs ~330us of matmul).  Each
slot emits: scores(g) [2 key blocks x 2 heads, row-packed K=64 matmuls],
PV(g-2) [lag-2 so exp latency is off the critical path], exp(g), plus paced
filler work (QKV projection for later head-pairs, output projection) to keep
the tensor engine ahead of the scalar engine.  Softmax normalization is
dripped through deferred DVE reciprocal halves + DRAM-bounce broadcast +
GPSIMD multiply, all off the matmul critical path.

Per-core device layout:
  x_sb[j]  [128, 2048] bf16   xT rows j*128.. (C x N), j = 0..7
  wq/wk/wv_sb[j] [128, 512]   W columns for this head group, per input-row j
  QT/KT[pr] [128, 2048] bf16  partition = dim-in-pair (2 heads x 64), free = tok
  VT[pr]   [128, 16, 2, 65]   partition = token-in-block; col 64 = ones (denom)
  OT[pr]   [128, 2048] bf16   normalized attention output, chan-pair x token
  out      [2048, 1024] bf16  partial projection output
"""

import functools
import sys

sys.path.insert(0, "/opt/trn_rl_repo")

from collections import deque
from contextlib import ExitStack

import numpy as np
from ml_dtypes import bfloat16

from concourse import bacc, mybir, tile
from concourse.bass_utils import run_bass_kernel_spmd

F32 = mybir.dt.float32
BF16 = mybir.dt.bfloat16
EXP = mybir.ActivationFunctionType.Exp
LN = mybir.ActivationFunctionType.Ln
CPY = mybir.ActivationFunctionType.Copy
ADD = mybir.AluOpType.add
MULT = mybir.AluOpType.mult

B, N, C, H, D = 4, 2048, 1024, 16, 64
SCALE = 0.125


def _build(dbg=False):
    nc = bacc.Bacc("TRN2", target_bir_lowering=False, debug=False)
    xT = nc.dram_tensor("xT", [1024, 2048], BF16, kind="ExternalInput").ap()
    wcat = nc.dram_tensor("wcat", [1024, 1536], BF16, kind="ExternalInput").ap()
    qbias = nc.dram_tensor("qb", [128, 4], F32, kind="ExternalInput").ap()
    kbias = nc.dram_tensor("kb", [128, 4], F32, kind="ExternalInput").ap()
    pw = nc.dram_tensor("pw", [512, 1024], BF16, kind="ExternalInput").ap()
    out = nc.dram_tensor("out", [2048, 1024], BF16, kind="ExternalOutput").ap()
    scratch = nc.dram_tensor("scratch", [32, 512], F32).ap()
    if dbg:
        dbgb = nc.dram_tensor("dbgb", [8, 2048], BF16,
                              kind="ExternalOutput").ap()
        dbgf = nc.dram_tensor("dbgf", [66, 512], F32,
                              kind="ExternalOutput").ap()

    with tile.TileContext(nc) as tc, ExitStack() as ctx:
        sb = ctx.enter_context(tc.tile_pool(name="sb", bufs=1))
        ps = ctx.enter_context(tc.tile_pool(name="ps", bufs=1, space="PSUM"))

        x_sb = [sb.tile([128, 2048], BF16, name=f"x{j}") for j in range(8)]
        wq_sb = [sb.tile([128, 512], BF16, name=f"wq{j}") for j in range(8)]
        wk_sb = [sb.tile([128, 512], BF16, name=f"wk{j}") for j in range(8)]
        wv_sb = [sb.tile([128, 512], BF16, name=f"wv{j}") for j in range(8)]
        pw_sb = sb.tile([128, 4, 1024], BF16, tag="pw")
        QT = [sb.tile([128, 2048], BF16, name=f"QT{p}") for p in range(4)]
        KT = [sb.tile([128, 2048], BF16, name=f"KT{p}") for p in range(4)]
        VT = [sb.tile([128, 16, 2, 65], BF16, name=f"VT{p}") for p in range(4)]
        OT = [sb.tile([128, 2048], BF16, name=f"OT{p}") for p in range(4)]
        qb_sb = sb.tile([128, 4], F32, tag="qb")
        kb_sb = sb.tile([128, 4], F32, tag="kb")
        zc = sb.tile([128, 16, 2, 1], F32, tag="zc")
        onec = sb.tile([128, 1], F32, tag="onec")

        # x on the scalar engine's DMA queue, weights on sync's: the two
        # streams transfer in parallel so the QKV prologue starts sooner.
        for j in range(8):
            nc.sync.dma_start(wq_sb[j][:], wcat[j * 128:(j + 1) * 128, 0:512])
            nc.scalar.dma_start(x_sb[j][:], xT[j * 128:(j + 1) * 128, :])
        nc.sync.dma_start(qb_sb[:], qbias[:])
        nc.sync.dma_start(kb_sb[:], kbias[:])
        for j in range(8):
            nc.sync.dma_start(wk_sb[j][:], wcat[j * 128:(j + 1) * 128, 512:1024])
        for j in range(8):
            nc.sync.dma_start(wv_sb[j][:], wcat[j * 128:(j + 1) * 128, 1024:1536])
        nc.vector.memset(zc[:], 0.0)
        nc.vector.memset(onec[:], 1.0)
        for pr in range(4):
            nc.sync.dma_start(pw_sb[:, pr, :], pw[pr * 128:(pr + 1) * 128, :])
            nc.vector.tensor_scalar(out=VT[pr][:, :, :, 64:65], in0=zc[:],
                                    scalar1=onec[:], scalar2=None, op0=ADD)

        def qkv_groups(pr, rings=("acc",)):
            """32 emitters: (owner_pr, pe_cycles, fn).  For fills (pr>0), K
            comes first (the next head-pair's first scores need the whole K
            panel the moment its attention starts), then Q for query-block 0,
            then V, then the remaining Q blocks."""
            rr = {"i": 0}

            def ring():
                rr["i"] += 1
                return rings[rr["i"] % len(rings)]

            def gq(nb):
                acc = ps.tile([128, 512], F32, tag=ring(), bufs=2, name="acc")
                for j in range(8):
                    nc.tensor.matmul(acc[:, 0:256],
                                     wq_sb[j][:, pr * 128:(pr + 1) * 128],
                                     x_sb[j][:, nb * 256:(nb + 1) * 256],
                                     start=(j == 0), stop=(j == 7))
                nc.vector.tensor_scalar(
                    out=QT[pr][:, nb * 256:(nb + 1) * 256], in0=acc[:, 0:256],
                    scalar1=qb_sb[:, pr:pr + 1], scalar2=None, op0=ADD)

            def gk(nb):
                acc = ps.tile([128, 512], F32, tag=ring(), bufs=2, name="acc")
                for j in range(8):
                    nc.tensor.matmul(acc[:, 0:256],
                                     wk_sb[j][:, pr * 128:(pr + 1) * 128],
                                     x_sb[j][:, nb * 256:(nb + 1) * 256],
                                     start=(j == 0), stop=(j == 7))
                nc.vector.tensor_scalar(
                    out=KT[pr][:, nb * 256:(nb + 1) * 256], in0=acc[:, 0:256],
                    scalar1=kb_sb[:, pr:pr + 1], scalar2=None, op0=ADD)

            def gv(t):
                acc = ps.tile([128, 512], F32, tag=ring(), bufs=2, name="acc")
                for j in range(8):
                    nc.tensor.matmul(acc[:, 0:128],
                                     x_sb[j][:, t * 128:(t + 1) * 128],
                                     wv_sb[j][:, pr * 128:(pr + 1) * 128],
                                     start=(j == 0), stop=(j == 7))
                nc.vector.tensor_copy(
                    out=VT[pr][:, t, :, 0:64],
                    in_=acc[:, 0:128].rearrange("p (h d) -> p h d", h=2))

            if pr == 0:
                # prologue: DMA-arrival order (wq+x, then wk, then wv)
                for nb in range(8):
                    yield pr, 2048, functools.partial(gq, nb)
                for nb in range(8):
                    yield pr, 2048, functools.partial(gk, nb)
                for t in range(16):
                    yield pr, 1024, functools.partial(gv, t)
            else:
                for nb in range(8):
                    yield pr, 2048, functools.partial(gk, nb)
                for nb in range(2):
                    yield pr, 2048, functools.partial(gq, nb)
                for t in range(16):
                    yield pr, 1024, functools.partial(gv, t)
                for nb in range(2, 8):
                    yield pr, 2048, functools.partial(gq, nb)

        def proj_groups(qb, tail=False):
            """8 emitters: output projection for query block qb.  In the tail
            (after the last exp) the scalar engine is idle: its Copy drains
            PSUM and both the acc and oaug rings are free; output DMAs split
            across two engine queues."""
            q0 = qb * 512
            for gi, (ns, co) in enumerate((n, c) for n in range(4)
                                          for c in range(2)):
                def gp(ns=ns, co=co, gi=gi):
                    ring = ("oaug" if (tail and gi % 2) else "acc")
                    pj = ps.tile([128, 512], F32, tag=ring, bufs=2, name="pj")
                    for pr4 in range(4):
                        nc.tensor.matmul(
                            pj[:],
                            OT[pr4][:, q0 + ns * 128:q0 + (ns + 1) * 128],
                            pw_sb[:, pr4, co * 512:(co + 1) * 512],
                            start=(pr4 == 0), stop=(pr4 == 3))
                    so = sb.tile([128, 512], BF16, tag="so", bufs=4, name="so")
                    if tail:
                        nc.scalar.activation(so[:], pj[:], CPY,
                                             bias=0.0, scale=1.0)
                    else:
                        nc.vector.tensor_copy(out=so[:], in_=pj[:])
                    eng = nc.scalar if (tail and gi % 2) else nc.sync
                    eng.dma_start(
                        out[q0 + ns * 128:q0 + (ns + 1) * 128,
                            co * 512:(co + 1) * 512], so[:])
                yield 9, 2048, gp

        # ---- paced fill + deferred-normalize machinery -------------------
        fills = deque()
        pending = deque()
        st8 = {"filled": 0, "target": 0}

        def pace():
            st8["target"] += 1550
            popped = 0
            while fills and st8["filled"] < st8["target"] and popped < 2:
                _, cyc, g = fills.popleft()
                g()
                st8["filled"] += cyc
                popped += 1

        def flush(owner_max):
            while fills and fills[0][0] <= owner_max:
                _, cyc, g = fills.popleft()
                g()
                st8["filled"] += cyc

        def emit_norm(pr, qb, oaug0, oaug1):
            """Free the PV accumulators via immediate SBUF copies, then defer
            [reciprocal halves -> DRAM-bounce broadcast -> GPSIMD multiply]
            into later pipeline slots.  The last block instead runs its
            reciprocal as exp(-ln d) on the then-idle scalar engine."""
            final = pr == 3 and qb == 3
            q0 = qb * 512
            osts, rcs, rbs = [], [], []
            for hh, oaug in ((0, oaug0), (1, oaug1)):
                ost = sb.tile([65, 512], F32, tag="ost", bufs=4, name="ost")
                nc.vector.tensor_copy(out=ost[:], in_=oaug[:])
                if dbg and pr == 0 and qb == 0 and hh == 0:
                    nc.sync.dma_start(dbgf[0:65, :], ost[:])
                osts.append(ost)
                rcs.append(sb.tile([1, 512], F32, tag="rc", bufs=4, name="rc"))
                rbs.append(sb.tile([64, 512], F32, tag="rb", bufs=4, name="rb"))

            def recip_piece(hh, half):
                row = pr * 8 + qb * 2 + hh
                lo, hi = half * 256, (half + 1) * 256
                nc.vector.reciprocal(rcs[hh][0:1, lo:hi], osts[hh][64:65, lo:hi])
                nc.sync.dma_start(scratch[row:row + 1, lo:hi],
                                  rcs[hh][0:1, lo:hi])

            def recip_act(hh):
                row = pr * 8 + qb * 2 + hh
                lns = sb.tile([1, 512], F32, tag="lns", bufs=2, name="lns")
                nc.scalar.activation(lns[:], osts[hh][64:65, :], LN,
                                     bias=0.0, scale=1.0)
                nc.scalar.activation(rcs[hh][0:1, :], lns[:], EXP,
                                     bias=0.0, scale=-1.0)
                nc.sync.dma_start(scratch[row:row + 1, :], rcs[hh][0:1, :])

            def bcast_piece():
                for hh in range(2):
                    row = pr * 8 + qb * 2 + hh
                    nc.sync.dma_start(
                        rbs[hh][:],
                        scratch[row:row + 1, :].to_broadcast((64, 512)))

            def norm_piece(hh):
                nc.gpsimd.tensor_tensor(out=OT[pr][hh * 64:(hh + 1) * 64,
                                                   q0:q0 + 512],
                                        in0=osts[hh][0:64, :], in1=rbs[hh][:],
                                        op=MULT)

            if final:
                recip_act(0)
                recip_act(1)
                bcast_piece()
                norm_piece(0)
                norm_piece(1)
            else:
                pending.extend([
                    lambda: recip_piece(0, 0), lambda: recip_piece(1, 0),
                    lambda: recip_piece(0, 1), lambda: recip_piece(1, 1),
                    bcast_piece,
                    lambda: norm_piece(0), lambda: norm_piece(1),
                ])

        # ---- prologue: QKV for head-pair 0 -------------------------------
        # Spread the partial j-accumulations over three PSUM rings so up to
        # six groups are in flight while the x tiles stream in.
        for _, _, g in qkv_groups(0, rings=("acc", "stage", "oaug")):
            g()
        if dbg:
            nc.sync.dma_start(dbgb[0:1, :], QT[0][0:1, :])
            nc.sync.dma_start(dbgb[1:2, :], KT[0][0:1, :])
            nc.sync.dma_start(dbgb[2:3, 0:65], VT[0][0:1, 0, 0, :])
            nc.sync.dma_start(dbgb[3:4, 0:65], VT[0][0:1, 0, 1, :])
        for p in (1, 2, 3):
            fills.extend(qkv_groups(p))

        # ---- seamless pipelined attention over all (pr, qb, tg) ----------
        stagedq = deque()
        oaug_map = {}

        def emit_pv(ent):
            pr, qb, tg, P0, P1, t0, t1 = ent
            if tg == 0:
                o0 = ps.tile([65, 512], F32, tag="oaug", bufs=2, name="oaug0")
                o1 = ps.tile([65, 512], F32, tag="oaug", bufs=2, name="oaug1")
                oaug_map[(pr, qb)] = (o0, o1)
            o0, o1 = oaug_map[(pr, qb)]
            st, sp = tg == 0, tg == 7
            nc.tensor.matmul(o0[:], VT[pr][:, t0, 0, :], P0[:, 0:512],
                             start=st, stop=False)
            nc.tensor.matmul(o0[:], VT[pr][:, t1, 0, :], P0[:, 512:1024],
                             start=False, stop=sp)
            nc.tensor.matmul(o1[:], VT[pr][:, t0, 1, :], P1[:, 0:512],
                             start=st, stop=False)
            nc.tensor.matmul(o1[:], VT[pr][:, t1, 1, :], P1[:, 512:1024],
                             start=False, stop=sp)
            if sp:
                emit_norm(pr, qb, o0, o1)
                del oaug_map[(pr, qb)]

        SLOTS = [(pr, qb, tg) for pr in range(4) for qb in range(4)
                 for tg in range(8)]
        for g, (pr, qb, tg) in enumerate(SLOTS):
            if qb == 0 and tg == 0:
                flush(pr)  # QKV(pr) must be fully emitted before its scores
            q0 = qb * 512
            t0, t1 = 2 * tg, 2 * tg + 1
            stage0 = ps.tile([128, 1024], F32, tag="stage", bufs=2,
                             name="stage0")
            stage1 = ps.tile([128, 1024], F32, tag="stage", bufs=2,
                             name="stage1")
            # scores S^T [keys, queries]; heads (2pr, 2pr+1) row-packed
            nc.tensor.matmul(stage0[:, 0:512],
                             KT[pr][0:64, t0 * 128:(t0 + 1) * 128],
                             QT[pr][0:64, q0:q0 + 512],
                             start=True, stop=True, tile_position=(0, 0))
            nc.tensor.matmul(stage1[:, 0:512],
                             KT[pr][64:128, t0 * 128:(t0 + 1) * 128],
                             QT[pr][64:128, q0:q0 + 512],
                             start=True, stop=True, tile_position=(64, 0))
            nc.tensor.matmul(stage0[:, 512:1024],
                             KT[pr][0:64, t1 * 128:(t1 + 1) * 128],
                             QT[pr][0:64, q0:q0 + 512],
                             start=True, stop=True, tile_position=(0, 0))
            nc.tensor.matmul(stage1[:, 512:1024],
                             KT[pr][64:128, t1 * 128:(t1 + 1) * 128],
                             QT[pr][64:128, q0:q0 + 512],
                             start=True, stop=True, tile_position=(64, 0))
            if len(stagedq) == 2:
                emit_pv(stagedq.popleft())
            P0 = sb.tile([128, 1024], BF16, tag="p", bufs=6, name="P0")
            P1 = sb.tile([128, 1024], BF16, tag="p", bufs=6, name="P1")
            nc.scalar.activation(P0[:], stage0[:], EXP, bias=0.0, scale=SCALE)
            nc.scalar.activation(P1[:], stage1[:], EXP, bias=0.0, scale=SCALE)
            if dbg and g == 0:
                nc.sync.dma_start(dbgb[4:5, 0:1024], P0[0:1, :])
                nc.sync.dma_start(dbgb[5:6, 0:1024], P1[0:1, :])
            stagedq.append((pr, qb, tg, P0, P1, t0, t1))
            if pr == 3 and tg == 6 and qb >= 1:
                fills.extend(proj_groups(qb - 1))
            pace()
            for _ in range(2 if pr == 3 else 1):
                if pending:
                    pending.popleft()()

        while stagedq:
            emit_pv(stagedq.popleft())
        while pending:
            pending.popleft()()
        while fills:
            fills.popleft()[2]()
        for _, _, g2 in proj_groups(3, tail=True):
            g2()
    return nc


def _prepare_in_maps(x, qkv_w, qkv_b, proj_w):
    x = np.asarray(x, dtype=np.float32)
    wb = np.asarray(qkv_w, dtype=np.float32).astype(bfloat16)
    pwb = np.asarray(proj_w, dtype=np.float32).astype(bfloat16)
    qkv_b = np.asarray(qkv_b, dtype=np.float32)
    in_maps = []
    for c in range(8):
        b, g = c % 4, c // 4
        w0 = 512 * g
        in_maps.append({
            "xT": np.ascontiguousarray(x[b].T).astype(bfloat16),
            "wcat": np.ascontiguousarray(np.concatenate(
                [wb[:, w0:w0 + 512],
                 wb[:, 1024 + w0:1024 + w0 + 512],
                 wb[:, 2048 + w0:2048 + w0 + 512]], axis=1)),
            "qb": np.ascontiguousarray(qkv_b[w0:w0 + 512].reshape(4, 128).T),
            "kb": np.ascontiguousarray(
                qkv_b[1024 + w0:1024 + w0 + 512].reshape(4, 128).T),
            "pw": np.ascontiguousarray(pwb[w0:w0 + 512, :]),
        })
    return in_maps


def _gather(parts, qkv_b, proj_w, proj_b):
    const_row = (np.asarray(qkv_b)[2048:].astype(np.float64)
                 @ np.asarray(proj_w).astype(np.float64)
                 + np.asarray(proj_b).astype(np.float64))
    out = np.empty((B, N, C), np.float32)
    for b in range(B):
        out[b] = (np.asarray(parts[b]).astype(np.float64)
                  + np.asarray(parts[b + 4]).astype(np.float64)
                  + const_row).astype(np.float32)
    return out


def kernel(**inputs: np.ndarray) -> np.ndarray:
    x = np.asarray(inputs["x"], dtype=np.float32)
    qkv_w = np.asarray(inputs["qkv_w"], dtype=np.float32)
    qkv_b = np.asarray(inputs["qkv_b"], dtype=np.float32)
    proj_w = np.asarray(inputs["proj_w"], dtype=np.float32)
    proj_b = np.asarray(inputs["proj_b"], dtype=np.float32)

    in_maps = _prepare_in_maps(x, qkv_w, qkv_b, proj_w)
    nc = _build()
    nc.finalize()
    res = run_bass_kernel_spmd(nc, in_maps, list(range(8)))
    parts = [res.results[c]["out"] for c in range(8)]
    return _gather(parts, qkv_b, proj_w, proj_b)


if __name__ == "__main__":
    import tempfile
    import time

    from concourse.bass_utils import compile_bass_kernel

    t0 = time.time()
    nc = _build()
    nc.compile()
    with tempfile.TemporaryDirectory() as td:
        compile_bass_kernel(nc, td, neff_name="k.neff")
    print(f"COMPILE OK ({time.time() - t0:.0f}s)", flush=True)


# revision 51
# speedup vs baseline: 1.3128x; 1.0123x over previous
"""Multi-head attention (B=4, N=2048, C=1024, H=16, D=64) on 8 TRN2 cores.

Sharding: core c -> batch b = c%4, head-group g = c//4 (local heads 0..7 are
global heads 8g..8g+7).  Each core computes its head group's contribution to
the output projection for its batch; host sums core b + core b+4 and adds
const_row = qkv_b[2048:] @ proj_w + proj_b (V-bias folds exactly through the
row-normalized attention: attn @ (1*bv^T) = 1*bv^T).

v3: all-bf16 datapath; one seamless software pipeline over every
(head-pair pr, query-block qb, key-block-pair tg) slot so the scores ->
exp(scalar engine) stream NEVER pauses at qb/pr boundaries (the exp stream is
the co-bottleneck: ~285us of scalar-engine work vs ~330us of matmul).  Each
slot emits: scores(g) [2 key blocks x 2 heads, row-packed K=64 matmuls],
PV(g-2) [lag-2 so exp latency is off the critical path], exp(g), plus paced
filler work (QKV projection for later head-pairs, output projection) to keep
the tensor engine ahead of the scalar engine.  Softmax normalization is
dripped through deferred DVE reciprocal halves + DRAM-bounce broadcast +
GPSIMD multiply, all off the matmul critical path.

Per-core device layout:
  x_sb[j]  [128, 2048] bf16   xT rows j*128.. (C x N), j = 0..7
  wq/wk/wv_sb[j] [128, 512]   W columns for this head group, per input-row j
  QT/KT[pr] [128, 2048] bf16  partition = dim-in-pair (2 heads x 64), free = tok
  VT[pr]   [128, 16, 2, 65]   partition = token-in-block; col 64 = ones (denom)
  OT[pr]   [128, 2048] bf16   normalized attention output, chan-pair x token
  out      [2048, 1024] bf16  partial projection output
"""

import functools
import sys

sys.path.insert(0, "/opt/trn_rl_repo")

from collections import deque
from contextlib import ExitStack

import numpy as np
from ml_dtypes import bfloat16

from concourse import bacc, mybir, tile
from concourse.bass_utils import run_bass_kernel_spmd

F32 = mybir.dt.float32
BF16 = mybir.dt.bfloat16
EXP = mybir.ActivationFunctionType.Exp
LN = mybir.ActivationFunctionType.Ln
CPY = mybir.ActivationFunctionType.Copy
ADD = mybir.AluOpType.add
MULT = mybir.AluOpType.mult

B, N, C, H, D = 4, 2048, 1024, 16, 64
SCALE = 0.125


def _build(dbg=False):
    nc = bacc.Bacc("TRN2", target_bir_lowering=False, debug=False)
    xT = nc.dram_tensor("xT", [1024, 2048], BF16, kind="ExternalInput").ap()
    wcat = nc.dram_tensor("wcat", [1024, 1536], BF16, kind="ExternalInput").ap()
    qbias = nc.dram_tensor("qb", [128, 4], F32, kind="ExternalInput").ap()
    kbias = nc.dram_tensor("kb", [128, 4], F32, kind="ExternalInput").ap()
    pw = nc.dram_tensor("pw", [512, 1024], BF16, kind="ExternalInput").ap()
    out = nc.dram_tensor("out", [2048, 1024], BF16, kind="ExternalOutput").ap()
    scratch = nc.dram_tensor("scratch", [32, 512], F32).ap()
    if dbg:
        dbgb = nc.dram_tensor("dbgb", [8, 2048], BF16,
                              kind="ExternalOutput").ap()
        dbgf = nc.dram_tensor("dbgf", [66, 512], F32,
                              kind="ExternalOutput").ap()

    with tile.TileContext(nc) as tc, ExitStack() as ctx:
        sb = ctx.enter_context(tc.tile_pool(name="sb", bufs=1))
        ps = ctx.enter_context(tc.tile_pool(name="ps", bufs=1, space="PSUM"))

        x_sb = [sb.tile([128, 2048], BF16, name=f"x{j}") for j in range(8)]
        wq_sb = [sb.tile([128, 512], BF16, name=f"wq{j}") for j in range(8)]
        wk_sb = [sb.tile([128, 512], BF16, name=f"wk{j}") for j in range(8)]
        wv_sb = [sb.tile([128, 512], BF16, name=f"wv{j}") for j in range(8)]
        pw_sb = sb.tile([128, 4, 1024], BF16, tag="pw")
        QT = [sb.tile([128, 2048], BF16, name=f"QT{p}") for p in range(4)]
        KT = [sb.tile([128, 2048], BF16, name=f"KT{p}") for p in range(4)]
        VT = [sb.tile([128, 16, 2, 65], BF16, name=f"VT{p}") for p in range(4)]
        OT = [sb.tile([128, 2048], BF16, name=f"OT{p}") for p in range(4)]
        qb_sb = sb.tile([128, 4], F32, tag="qb")
        kb_sb = sb.tile([128, 4], F32, tag="kb")
        zc = sb.tile([128, 16, 2, 1], F32, tag="zc")
        onec = sb.tile([128, 1], F32, tag="onec")

        for j in range(8):
            nc.sync.dma_start(wq_sb[j][:], wcat[j * 128:(j + 1) * 128, 0:512])
            nc.sync.dma_start(x_sb[j][:], xT[j * 128:(j + 1) * 128, :])
        nc.sync.dma_start(qb_sb[:], qbias[:])
        nc.sync.dma_start(kb_sb[:], kbias[:])
        for j in range(8):
            nc.sync.dma_start(wk_sb[j][:], wcat[j * 128:(j + 1) * 128, 512:1024])
        for j in range(8):
            nc.sync.dma_start(wv_sb[j][:], wcat[j * 128:(j + 1) * 128, 1024:1536])
        nc.vector.memset(zc[:], 0.0)
        nc.vector.memset(onec[:], 1.0)
        for pr in range(4):
            nc.sync.dma_start(pw_sb[:, pr, :], pw[pr * 128:(pr + 1) * 128, :])
            nc.vector.tensor_scalar(out=VT[pr][:, :, :, 64:65], in0=zc[:],
                                    scalar1=onec[:], scalar2=None, op0=ADD)

        def qkv_groups(pr, rings=("acc",)):
            """32 emitters: (owner_pr, pe_cycles, fn).  For fills (pr>0), K
            comes first (the next head-pair's first scores need the whole K
            panel the moment its attention starts), then Q for query-block 0,
            then V, then the remaining Q blocks."""
            rr = {"i": 0}

            def ring():
                rr["i"] += 1
                return rings[rr["i"] % len(rings)]

            def gq(nb):
                acc = ps.tile([128, 512], F32, tag=ring(), bufs=2, name="acc")
                for j in range(8):
                    nc.tensor.matmul(acc[:, 0:256],
                                     wq_sb[j][:, pr * 128:(pr + 1) * 128],
                                     x_sb[j][:, nb * 256:(nb + 1) * 256],
                                     start=(j == 0), stop=(j == 7))
                nc.vector.tensor_scalar(
                    out=QT[pr][:, nb * 256:(nb + 1) * 256], in0=acc[:, 0:256],
                    scalar1=qb_sb[:, pr:pr + 1], scalar2=None, op0=ADD)

            def gk(nb):
                acc = ps.tile([128, 512], F32, tag=ring(), bufs=2, name="acc")
                for j in range(8):
                    nc.tensor.matmul(acc[:, 0:256],
                                     wk_sb[j][:, pr * 128:(pr + 1) * 128],
                                     x_sb[j][:, nb * 256:(nb + 1) * 256],
                                     start=(j == 0), stop=(j == 7))
                nc.vector.tensor_scalar(
                    out=KT[pr][:, nb * 256:(nb + 1) * 256], in0=acc[:, 0:256],
                    scalar1=kb_sb[:, pr:pr + 1], scalar2=None, op0=ADD)

            def gv(t):
                acc = ps.tile([128, 512], F32, tag=ring(), bufs=2, name="acc")
                for j in range(8):
                    nc.tensor.matmul(acc[:, 0:128],
                                     x_sb[j][:, t * 128:(t + 1) * 128],
                                     wv_sb[j][:, pr * 128:(pr + 1) * 128],
                                     start=(j == 0), stop=(j == 7))
                nc.vector.tensor_copy(
                    out=VT[pr][:, t, :, 0:64],
                    in_=acc[:, 0:128].rearrange("p (h d) -> p h d", h=2))

            if pr == 0:
                # prologue: DMA-arrival order (wq+x, then wk, then wv)
                for nb in range(8):
                    yield pr, 2048, functools.partial(gq, nb)
                for nb in range(8):
                    yield pr, 2048, functools.partial(gk, nb)
                for t in range(16):
                    yield pr, 1024, functools.partial(gv, t)
            else:
                # K then Q then V: the first scores of attn(pr) have a
                # whole-tile dependency on QT/KT, so those must land early;
                # V is only needed by PV, two slots later.
                for nb in range(8):
                    yield pr, 2048, functools.partial(gk, nb)
                for nb in range(8):
                    yield pr, 2048, functools.partial(gq, nb)
                for t in range(16):
                    yield pr, 1024, functools.partial(gv, t)

        def proj_groups(qb, tail=False):
            """8 emitters: output projection for query block qb.  In the tail
            (after the last exp) the scalar engine is idle: its Copy drains
            PSUM and both the acc and oaug rings are free; output DMAs split
            across two engine queues."""
            q0 = qb * 512
            for gi, (ns, co) in enumerate((n, c) for n in range(4)
                                          for c in range(2)):
                def gp(ns=ns, co=co, gi=gi):
                    ring = ("oaug" if (tail and gi % 2) else "acc")
                    pj = ps.tile([128, 512], F32, tag=ring, bufs=2, name="pj")
                    for pr4 in range(4):
                        nc.tensor.matmul(
                            pj[:],
                            OT[pr4][:, q0 + ns * 128:q0 + (ns + 1) * 128],
                            pw_sb[:, pr4, co * 512:(co + 1) * 512],
                            start=(pr4 == 0), stop=(pr4 == 3))
                    so = sb.tile([128, 512], BF16, tag="so", bufs=4, name="so")
                    # alternate the PSUM-draining copy between the scalar
                    # engine (which has slack during head-pair 3) and the DVE
                    if gi % 2:
                        nc.scalar.activation(so[:], pj[:], CPY,
                                             bias=0.0, scale=1.0)
                    else:
                        nc.vector.tensor_copy(out=so[:], in_=pj[:])
                    eng = nc.scalar if (tail and gi % 2) else nc.sync
                    eng.dma_start(
                        out[q0 + ns * 128:q0 + (ns + 1) * 128,
                            co * 512:(co + 1) * 512], so[:])
                yield 9, 2048, gp

        # ---- paced fill + deferred-normalize machinery -------------------
        fills = deque()
        pending = deque()
        st8 = {"filled": 0, "target": 0}

        def pace():
            st8["target"] += 1550
            popped = 0
            while fills and st8["filled"] < st8["target"] and popped < 2:
                _, cyc, g = fills.popleft()
                g()
                st8["filled"] += cyc
                popped += 1

        def flush(owner_max):
            while fills and fills[0][0] <= owner_max:
                _, cyc, g = fills.popleft()
                g()
                st8["filled"] += cyc

        def emit_norm(pr, qb, oaug0, oaug1):
            """Free the PV accumulators via immediate SBUF copies, then defer
            [reciprocal halves -> DRAM-bounce broadcast -> GPSIMD multiply]
            into later pipeline slots.  The last block instead runs its
            reciprocal as exp(-ln d) on the then-idle scalar engine."""
            final = pr == 3 and qb == 3
            q0 = qb * 512
            osts, rcs, rbs = [], [], []
            for hh, oaug in ((0, oaug0), (1, oaug1)):
                ost = sb.tile([65, 512], F32, tag="ost", bufs=4, name="ost")
                nc.vector.tensor_copy(out=ost[:], in_=oaug[:])
                if dbg and pr == 0 and qb == 0 and hh == 0:
                    nc.sync.dma_start(dbgf[0:65, :], ost[:])
                osts.append(ost)
                rcs.append(sb.tile([1, 512], F32, tag="rc", bufs=4, name="rc"))
                rbs.append(sb.tile([64, 512], F32, tag="rb", bufs=4, name="rb"))

            def recip_piece(hh, half):
                row = pr * 8 + qb * 2 + hh
                lo, hi = half * 256, (half + 1) * 256
                nc.vector.reciprocal(rcs[hh][0:1, lo:hi], osts[hh][64:65, lo:hi])
                nc.sync.dma_start(scratch[row:row + 1, lo:hi],
                                  rcs[hh][0:1, lo:hi])

            def recip_act(hh):
                row = pr * 8 + qb * 2 + hh
                lns = sb.tile([1, 512], F32, tag="lns", bufs=2, name="lns")
                nc.scalar.activation(lns[:], osts[hh][64:65, :], LN,
                                     bias=0.0, scale=1.0)
                nc.scalar.activation(rcs[hh][0:1, :], lns[:], EXP,
                                     bias=0.0, scale=-1.0)
                nc.sync.dma_start(scratch[row:row + 1, :], rcs[hh][0:1, :])

            def bcast_piece():
                for hh in range(2):
                    row = pr * 8 + qb * 2 + hh
                    nc.sync.dma_start(
                        rbs[hh][:],
                        scratch[row:row + 1, :].to_broadcast((64, 512)))

            def norm_piece(hh):
                nc.gpsimd.tensor_tensor(out=OT[pr][hh * 64:(hh + 1) * 64,
                                                   q0:q0 + 512],
                                        in0=osts[hh][0:64, :], in1=rbs[hh][:],
                                        op=MULT)

            if pr == 3:
                # scalar engine has slack in head-pair 3: run 1/d = exp(-ln d)
                # there inline, keeping the DVE free for PSUM recycling
                recip_act(0)
                recip_act(1)
                bcast_piece()
                norm_piece(0)
                norm_piece(1)
            else:
                pending.extend([
                    lambda: recip_piece(0, 0), lambda: recip_piece(1, 0),
                    lambda: recip_piece(0, 1), lambda: recip_piece(1, 1),
                    bcast_piece,
                    lambda: norm_piece(0), lambda: norm_piece(1),
                ])

        # ---- prologue: QKV for head-pair 0 -------------------------------
        # Spread the partial j-accumulations over three PSUM rings so up to
        # six groups are in flight while the x tiles stream in.
        for _, _, g in qkv_groups(0, rings=("acc", "stage", "oaug")):
            g()
        if dbg:
            nc.sync.dma_start(dbgb[0:1, :], QT[0][0:1, :])
            nc.sync.dma_start(dbgb[1:2, :], KT[0][0:1, :])
            nc.sync.dma_start(dbgb[2:3, 0:65], VT[0][0:1, 0, 0, :])
            nc.sync.dma_start(dbgb[3:4, 0:65], VT[0][0:1, 0, 1, :])
        for p in (1, 2, 3):
            fills.extend(qkv_groups(p))

        # ---- seamless pipelined attention over all (pr, qb, tg) ----------
        stagedq = deque()
        oaug_map = {}

        def emit_pv(ent):
            pr, qb, tg, P0, P1, t0, t1 = ent
            if tg == 0:
                o0 = ps.tile([65, 512], F32, tag="oaug", bufs=2, name="oaug0")
                o1 = ps.tile([65, 512], F32, tag="oaug", bufs=2, name="oaug1")
                oaug_map[(pr, qb)] = (o0, o1)
            o0, o1 = oaug_map[(pr, qb)]
            st, sp = tg == 0, tg == 7
            nc.tensor.matmul(o0[:], VT[pr][:, t0, 0, :], P0[:, 0:512],
                             start=st, stop=False)
            nc.tensor.matmul(o0[:], VT[pr][:, t1, 0, :], P0[:, 512:1024],
                             start=False, stop=sp)
            nc.tensor.matmul(o1[:], VT[pr][:, t0, 1, :], P1[:, 0:512],
                             start=st, stop=False)
            nc.tensor.matmul(o1[:], VT[pr][:, t1, 1, :], P1[:, 512:1024],
                             start=False, stop=sp)
            if sp:
                emit_norm(pr, qb, o0, o1)
                del oaug_map[(pr, qb)]

        SLOTS = [(pr, qb, tg) for pr in range(4) for qb in range(4)
                 for tg in range(8)]
        for g, (pr, qb, tg) in enumerate(SLOTS):
            if qb == 0 and tg == 0:
                flush(pr)  # QKV(pr) must be fully emitted before its scores
            q0 = qb * 512
            t0, t1 = 2 * tg, 2 * tg + 1
            stage0 = ps.tile([128, 1024], F32, tag="stage", bufs=2,
                             name="stage0")
            stage1 = ps.tile([128, 1024], F32, tag="stage", bufs=2,
                             name="stage1")
            # scores S^T [keys, queries]; heads (2pr, 2pr+1) row-packed
            nc.tensor.matmul(stage0[:, 0:512],
                             KT[pr][0:64, t0 * 128:(t0 + 1) * 128],
                             QT[pr][0:64, q0:q0 + 512],
                             start=True, stop=True, tile_position=(0, 0))
            nc.tensor.matmul(stage1[:, 0:512],
                             KT[pr][64:128, t0 * 128:(t0 + 1) * 128],
                             QT[pr][64:128, q0:q0 + 512],
                             start=True, stop=True, tile_position=(64, 0))
            nc.tensor.matmul(stage0[:, 512:1024],
                             KT[pr][0:64, t1 * 128:(t1 + 1) * 128],
                             QT[pr][0:64, q0:q0 + 512],
                             start=True, stop=True, tile_position=(0, 0))
            nc.tensor.matmul(stage1[:, 512:1024],
                             KT[pr][64:128, t1 * 128:(t1 + 1) * 128],
                             QT[pr][64:128, q0:q0 + 512],
                             start=True, stop=True, tile_position=(64, 0))
            if len(stagedq) == 2:
                emit_pv(stagedq.popleft())
            P0 = sb.tile([128, 1024], BF16, tag="p", bufs=6, name="P0")
            P1 = sb.tile([128, 1024], BF16, tag="p", bufs=6, name="P1")
            nc.scalar.activation(P0[:], stage0[:], EXP, bias=0.0, scale=SCALE)
            nc.scalar.activation(P1[:], stage1[:], EXP, bias=0.0, scale=SCALE)
            if dbg and g == 0:
                nc.sync.dma_start(dbgb[4:5, 0:1024], P0[0:1, :])
                nc.sync.dma_start(dbgb[5:6, 0:1024], P1[0:1, :])
            stagedq.append((pr, qb, tg, P0, P1, t0, t1))
            if pr == 3 and tg == 6 and qb >= 1:
                fills.extend(proj_groups(qb - 1))
            pace()
            for _ in range(2 if pr == 3 else 1):
                if pending:
                    pending.popleft()()

        while stagedq:
            emit_pv(stagedq.popleft())
        while pending:
            pending.popleft()()
        while fills:
            fills.popleft()[2]()
        for _, _, g2 in proj_groups(3, tail=True):
            g2()
    return nc


def _prepare_in_maps(x, qkv_w, qkv_b, proj_w):
    x = np.asarray(x, dtype=np.float32)
    wb = np.asarray(qkv_w, dtype=np.float32).astype(bfloat16)
    pwb = np.asarray(proj_w, dtype=np.float32).astype(bfloat16)
    qkv_b = np.asarray(qkv_b, dtype=np.float32)
    in_maps = []
    for c in range(8):
        b, g = c % 4, c // 4
        w0 = 512 * g
        in_maps.append({
            "xT": np.ascontiguousarray(x[b].T).astype(bfloat16),
            "wcat": np.ascontiguousarray(np.concatenate(
                [wb[:, w0:w0 + 512],
                 wb[:, 1024 + w0:1024 + w0 + 512],
                 wb[:, 2048 + w0:2048 + w0 + 512]], axis=1)),
            "qb": np.ascontiguousarray(qkv_b[w0:w0 + 512].reshape(4, 128).T),
            "kb": np.ascontiguousarray(
                qkv_b[1024 + w0:1024 + w0 + 512].reshape(4, 128).T),
            "pw": np.ascontiguousarray(pwb[w0:w0 + 512, :]),
        })
    return in_maps


def _gather(parts, qkv_b, proj_w, proj_b):
    const_row = (np.asarray(qkv_b)[2048:].astype(np.float64)
                 @ np.asarray(proj_w).astype(np.float64)
                 + np.asarray(proj_b).astype(np.float64))
    out = np.empty((B, N, C), np.float32)
    for b in range(B):
        out[b] = (np.asarray(parts[b]).astype(np.float64)
                  + np.asarray(parts[b + 4]).astype(np.float64)
                  + const_row).astype(np.float32)
    return out


def kernel(**inputs: np.ndarray) -> np.ndarray:
    x = np.asarray(inputs["x"], dtype=np.float32)
    qkv_w = np.asarray(inputs["qkv_w"], dtype=np.float32)
    qkv_b = np.asarray(inputs["qkv_b"], dtype=np.float32)
    proj_w = np.asarray(inputs["proj_w"], dtype=np.float32)
    proj_b = np.asarray(inputs["proj_b"], dtype=np.float32)

    in_maps = _prepare_in_maps(x, qkv_w, qkv_b, proj_w)
    nc = _build()
    nc.finalize()
    res = run_bass_kernel_spmd(nc, in_maps, list(range(8)))
    parts = [res.results[c]["out"] for c in range(8)]
    return _gather(parts, qkv_b, proj_w, proj_b)


if __name__ == "__main__":
    import tempfile
    import time

    from concourse.bass_utils import compile_bass_kernel

    t0 = time.time()
    nc = _build()
    nc.compile()
    with tempfile.TemporaryDirectory() as td:
        compile_bass_kernel(nc, td, neff_name="k.neff")
    print(f"COMPILE OK ({time.time() - t0:.0f}s)", flush=True)


# revision 57
# speedup vs baseline: 1.3438x; 1.0236x over previous
"""Multi-head attention (B=4, N=2048, C=1024, H=16, D=64) on 8 TRN2 cores.

Sharding: core c -> batch b = c%4, head-group g = c//4 (local heads 0..7 are
global heads 8g..8g+7).  Each core computes its head group's contribution to
the output projection for its batch; host sums core b + core b+4 and adds
const_row = qkv_b[2048:] @ proj_w + proj_b (V-bias folds exactly through the
row-normalized attention: attn @ (1*bv^T) = 1*bv^T).

v3: all-bf16 datapath; one seamless software pipeline over every
(head-pair pr, query-block qb, key-block-pair tg) slot so the scores ->
exp(scalar engine) stream NEVER pauses at qb/pr boundaries (the exp stream is
the co-bottleneck: ~285us of scalar-engine work vs ~330us of matmul).  Each
slot emits: scores(g) [2 key blocks x 2 heads, row-packed K=64 matmuls],
PV(g-2) [lag-2 so exp latency is off the critical path], exp(g), plus paced
filler work (QKV projection for later head-pairs, output projection) to keep
the tensor engine ahead of the scalar engine.  Softmax normalization is
dripped through deferred DVE reciprocal halves + DRAM-bounce broadcast +
GPSIMD multiply, all off the matmul critical path.

Per-core device layout:
  x_sb[j]  [128, 2048] bf16   xT rows j*128.. (C x N), j = 0..7
  wq/wk/wv_sb[j] [128, 512]   W columns for this head group, per input-row j
  QT/KT[pr] [128, 2048] bf16  partition = dim-in-pair (2 heads x 64), free = tok
  VT[pr]   [128, 16, 2, 65]   partition = token-in-block; col 64 = ones (denom)
  OT[pr]   [128, 2048] bf16   normalized attention output, chan-pair x token
  out      [2048, 1024] bf16  partial projection output
"""

import functools
import sys

sys.path.insert(0, "/opt/trn_rl_repo")

from collections import deque
from contextlib import ExitStack

import numpy as np
from ml_dtypes import bfloat16

from concourse import bacc, mybir, tile
from concourse.bass_utils import run_bass_kernel_spmd

F32 = mybir.dt.float32
BF16 = mybir.dt.bfloat16
EXP = mybir.ActivationFunctionType.Exp
LN = mybir.ActivationFunctionType.Ln
CPY = mybir.ActivationFunctionType.Copy
ADD = mybir.AluOpType.add
MULT = mybir.AluOpType.mult

B, N, C, H, D = 4, 2048, 1024, 16, 64
SCALE = 0.125


def _build(dbg=False):
    nc = bacc.Bacc("TRN2", target_bir_lowering=False, debug=False)
    xT = nc.dram_tensor("xT", [1024, 2048], BF16, kind="ExternalInput").ap()
    wcat = nc.dram_tensor("wcat", [1024, 1536], BF16, kind="ExternalInput").ap()
    qbias = nc.dram_tensor("qb", [128, 4], F32, kind="ExternalInput").ap()
    kbias = nc.dram_tensor("kb", [128, 4], F32, kind="ExternalInput").ap()
    pw = nc.dram_tensor("pw", [512, 1024], BF16, kind="ExternalInput").ap()
    out = nc.dram_tensor("out", [2048, 1024], BF16, kind="ExternalOutput").ap()
    scratch = nc.dram_tensor("scratch", [32, 512], F32).ap()
    if dbg:
        dbgb = nc.dram_tensor("dbgb", [8, 2048], BF16,
                              kind="ExternalOutput").ap()
        dbgf = nc.dram_tensor("dbgf", [66, 512], F32,
                              kind="ExternalOutput").ap()

    with tile.TileContext(nc) as tc, ExitStack() as ctx:
        sb = ctx.enter_context(tc.tile_pool(name="sb", bufs=1))
        ps = ctx.enter_context(tc.tile_pool(name="ps", bufs=1, space="PSUM"))

        x_sb = [sb.tile([128, 2048], BF16, name=f"x{j}") for j in range(8)]
        wq_sb = [sb.tile([128, 512], BF16, name=f"wq{j}") for j in range(8)]
        wk_sb = [sb.tile([128, 512], BF16, name=f"wk{j}") for j in range(8)]
        wv_sb = [sb.tile([128, 512], BF16, name=f"wv{j}") for j in range(8)]
        pw_sb = sb.tile([128, 4, 1024], BF16, tag="pw")
        QT = [[sb.tile([128, 1024], BF16, name=f"QT{p}h{h}") for h in (0, 1)]
              for p in range(4)]
        KT = [sb.tile([128, 2048], BF16, name=f"KT{p}") for p in range(4)]
        VT = [[sb.tile([128, 8, 2, 65], BF16, name=f"VT{p}h{h}") for h in (0, 1)]
              for p in range(4)]
        OT = [sb.tile([128, 2048], BF16, name=f"OT{p}") for p in range(4)]
        qb_sb = sb.tile([128, 4], F32, tag="qb")
        kb_sb = sb.tile([128, 4], F32, tag="kb")
        zc = sb.tile([128, 8, 2, 1], F32, tag="zc")
        onec = sb.tile([128, 1], F32, tag="onec")

        for j in range(8):
            nc.sync.dma_start(wq_sb[j][:], wcat[j * 128:(j + 1) * 128, 0:512])
            nc.sync.dma_start(x_sb[j][:], xT[j * 128:(j + 1) * 128, :])
        nc.sync.dma_start(qb_sb[:], qbias[:])
        nc.sync.dma_start(kb_sb[:], kbias[:])
        for j in range(8):
            nc.sync.dma_start(wk_sb[j][:], wcat[j * 128:(j + 1) * 128, 512:1024])
        for j in range(8):
            nc.sync.dma_start(wv_sb[j][:], wcat[j * 128:(j + 1) * 128, 1024:1536])
        nc.vector.memset(zc[:], 0.0)
        nc.vector.memset(onec[:], 1.0)
        for pr in range(4):
            nc.sync.dma_start(pw_sb[:, pr, :], pw[pr * 128:(pr + 1) * 128, :])
            for h in (0, 1):
                nc.vector.tensor_scalar(out=VT[pr][h][:, :, :, 64:65],
                                        in0=zc[:], scalar1=onec[:],
                                        scalar2=None, op0=ADD)

        def qkv_groups(pr, rings=("acc",)):
            """32 emitters: (owner_pr, pe_cycles, fn).  For fills (pr>0), K
            comes first (the next head-pair's first scores need the whole K
            panel the moment its attention starts), then Q for query-block 0,
            then V, then the remaining Q blocks."""
            ppr = max(pr, 0)
            rr = {"i": 0}

            def ring():
                rr["i"] += 1
                return rings[rr["i"] % len(rings)]

            def gq(nb):
                done.add(("q", ppr, nb))
                acc = ps.tile([128, 512], F32, tag=ring(), bufs=2, name="acc")
                for j in range(8):
                    nc.tensor.matmul(acc[:, 0:256],
                                     wq_sb[j][:, ppr * 128:(ppr + 1) * 128],
                                     x_sb[j][:, nb * 256:(nb + 1) * 256],
                                     start=(j == 0), stop=(j == 7))
                nc.vector.tensor_scalar(
                    out=QT[ppr][nb // 4][:, (nb % 4) * 256:(nb % 4 + 1) * 256],
                    in0=acc[:, 0:256],
                    scalar1=qb_sb[:, ppr:ppr + 1], scalar2=None, op0=ADD)

            def gk(nb):
                done.add(("k", ppr, nb))
                acc = ps.tile([128, 512], F32, tag=ring(), bufs=2, name="acc")
                for j in range(8):
                    nc.tensor.matmul(acc[:, 0:256],
                                     wk_sb[j][:, ppr * 128:(ppr + 1) * 128],
                                     x_sb[j][:, nb * 256:(nb + 1) * 256],
                                     start=(j == 0), stop=(j == 7))
                nc.vector.tensor_scalar(
                    out=KT[ppr][:, nb * 256:(nb + 1) * 256], in0=acc[:, 0:256],
                    scalar1=kb_sb[:, ppr:ppr + 1], scalar2=None, op0=ADD)

            def gv(t):
                done.add(("v", ppr, t))
                acc = ps.tile([128, 512], F32, tag=ring(), bufs=2, name="acc")
                for j in range(8):
                    nc.tensor.matmul(acc[:, 0:128],
                                     x_sb[j][:, t * 128:(t + 1) * 128],
                                     wv_sb[j][:, ppr * 128:(ppr + 1) * 128],
                                     start=(j == 0), stop=(j == 7))
                nc.vector.tensor_copy(
                    out=VT[ppr][t // 8][:, t % 8, :, 0:64],
                    in_=acc[:, 0:128].rearrange("p (h d) -> p h d", h=2))

            if pr == 0:
                # serial prologue: just enough for attention to start
                # (K panel, first Q half-panel, first V half-panel); the
                # rest streams in as paced fills
                for nb in range(4):
                    yield pr, 2048, functools.partial(gq, nb)
                for nb in range(8):
                    yield pr, 2048, functools.partial(gk, nb)
                for t in range(8):
                    yield pr, 1024, functools.partial(gv, t)
            elif pr == -1:
                # pr0 leftovers, deadline-ordered for the early slots
                for t in range(8, 16):
                    yield 0, 1024, functools.partial(gv, t)
                for nb in range(4, 8):
                    yield 0, 2048, functools.partial(gq, nb)
            else:
                for nb in range(8):
                    yield pr, 2048, functools.partial(gk, nb)
                for nb in range(4):
                    yield pr, 2048, functools.partial(gq, nb)
                for t in range(8):
                    yield pr, 1024, functools.partial(gv, t)
                for nb in range(4, 8):
                    yield pr, 2048, functools.partial(gq, nb)
                for t in range(8, 16):
                    yield pr, 1024, functools.partial(gv, t)

        def proj_groups(qb, tail=False):
            """8 emitters: output projection for query block qb.  In the tail
            (after the last exp) the scalar engine is idle: its Copy drains
            PSUM and both the acc and oaug rings are free; output DMAs split
            across two engine queues."""
            q0 = qb * 512
            for gi, (ns, co) in enumerate((n, c) for n in range(4)
                                          for c in range(2)):
                def gp(ns=ns, co=co, gi=gi):
                    ring = ("oaug" if (tail and gi % 2) else "acc")
                    pj = ps.tile([128, 512], F32, tag=ring, bufs=2, name="pj")
                    for pr4 in range(4):
                        nc.tensor.matmul(
                            pj[:],
                            OT[pr4][:, q0 + ns * 128:q0 + (ns + 1) * 128],
                            pw_sb[:, pr4, co * 512:(co + 1) * 512],
                            start=(pr4 == 0), stop=(pr4 == 3))
                    so = sb.tile([128, 512], BF16, tag="so", bufs=4, name="so")
                    # alternate the PSUM-draining copy between the scalar
                    # engine (which has slack during head-pair 3) and the DVE
                    if gi % 2:
                        nc.scalar.activation(so[:], pj[:], CPY,
                                             bias=0.0, scale=1.0)
                    else:
                        nc.vector.tensor_copy(out=so[:], in_=pj[:])
                    eng = nc.scalar if (tail and gi % 2) else nc.sync
                    eng.dma_start(
                        out[q0 + ns * 128:q0 + (ns + 1) * 128,
                            co * 512:(co + 1) * 512], so[:])
                yield 9, 2048, gp

        done = set()

        # ---- paced fill + deferred-normalize machinery -------------------
        fills = deque()
        pending = deque()
        st8 = {"filled": 0, "target": 0}

        def pace():
            st8["target"] += 1550
            popped = 0
            while fills and st8["filled"] < st8["target"] and popped < 2:
                _, cyc, g = fills.popleft()
                g()
                st8["filled"] += cyc
                popped += 1

        def flush(owner_max):
            while fills and fills[0][0] <= owner_max:
                _, cyc, g = fills.popleft()
                g()
                st8["filled"] += cyc

        def need(kind, pr, idx):
            # just-in-time: pop fills (they are deadline-ordered) until the
            # required QKV panel write has been emitted
            while (kind, pr, idx) not in done and fills:
                _, cyc, g = fills.popleft()
                g()
                st8["filled"] += cyc

        def emit_norm(pr, qb, oaug0, oaug1):
            """Free the PV accumulators via immediate SBUF copies, then defer
            [reciprocal halves -> DRAM-bounce broadcast -> GPSIMD multiply]
            into later pipeline slots.  The last block instead runs its
            reciprocal as exp(-ln d) on the then-idle scalar engine."""
            final = pr == 3 and qb == 3
            q0 = qb * 512
            osts, rcs, rbs = [], [], []
            for hh, oaug in ((0, oaug0), (1, oaug1)):
                ost = sb.tile([65, 512], F32, tag="ost", bufs=4, name="ost")
                nc.vector.tensor_copy(out=ost[:], in_=oaug[:])
                if dbg and pr == 0 and qb == 0 and hh == 0:
                    nc.sync.dma_start(dbgf[0:65, :], ost[:])
                osts.append(ost)
                rcs.append(sb.tile([1, 512], F32, tag="rc", bufs=4, name="rc"))
                rbs.append(sb.tile([64, 512], F32, tag="rb", bufs=4, name="rb"))

            def recip_piece(hh, half):
                row = pr * 8 + qb * 2 + hh
                lo, hi = half * 256, (half + 1) * 256
                nc.vector.reciprocal(rcs[hh][0:1, lo:hi], osts[hh][64:65, lo:hi])
                nc.sync.dma_start(scratch[row:row + 1, lo:hi],
                                  rcs[hh][0:1, lo:hi])

            def recip_act(hh):
                row = pr * 8 + qb * 2 + hh
                lns = sb.tile([1, 512], F32, tag="lns", bufs=2, name="lns")
                nc.scalar.activation(lns[:], osts[hh][64:65, :], LN,
                                     bias=0.0, scale=1.0)
                nc.scalar.activation(rcs[hh][0:1, :], lns[:], EXP,
                                     bias=0.0, scale=-1.0)
                nc.sync.dma_start(scratch[row:row + 1, :], rcs[hh][0:1, :])

            def bcast_piece():
                for hh in range(2):
                    row = pr * 8 + qb * 2 + hh
                    nc.sync.dma_start(
                        rbs[hh][:],
                        scratch[row:row + 1, :].to_broadcast((64, 512)))

            def norm_piece(hh):
                nc.gpsimd.tensor_tensor(out=OT[pr][hh * 64:(hh + 1) * 64,
                                                   q0:q0 + 512],
                                        in0=osts[hh][0:64, :], in1=rbs[hh][:],
                                        op=MULT)

            if final:
                # all exps are done: run 1/d = exp(-ln d) on the idle scalar
                # engine inline
                recip_act(0)
                recip_act(1)
                bcast_piece()
                norm_piece(0)
                norm_piece(1)
            else:
                pending.extend([
                    lambda: recip_piece(0, 0), lambda: recip_piece(1, 0),
                    lambda: recip_piece(0, 1), lambda: recip_piece(1, 1),
                    bcast_piece,
                    lambda: norm_piece(0), lambda: norm_piece(1),
                ])

        # ---- prologue: QKV for head-pair 0 -------------------------------
        # Spread the partial j-accumulations over three PSUM rings so up to
        # six groups are in flight while the x tiles stream in.
        for _, _, g in qkv_groups(0, rings=("acc", "stage", "oaug")):
            g()
        fills.extend(qkv_groups(-1))
        if dbg:
            nc.sync.dma_start(dbgb[0:1, 0:1024], QT[0][0][0:1, :])
            nc.sync.dma_start(dbgb[1:2, :], KT[0][0:1, :])
            nc.sync.dma_start(dbgb[2:3, 0:65], VT[0][0][0:1, 0, 0, :])
            nc.sync.dma_start(dbgb[3:4, 0:65], VT[0][0][0:1, 0, 1, :])
        for p in (1, 2, 3):
            fills.extend(qkv_groups(p))

        # ---- seamless pipelined attention over all (pr, qb, tg) ----------
        stagedq = deque()
        oaug_map = {}

        def emit_pv(ent):
            pr, qb, tg, P0, P1, t0, t1 = ent
            if tg == 0:
                o0 = ps.tile([65, 512], F32, tag="oaug", bufs=2, name="oaug0")
                o1 = ps.tile([65, 512], F32, tag="oaug", bufs=2, name="oaug1")
                oaug_map[(pr, qb)] = (o0, o1)
            o0, o1 = oaug_map[(pr, qb)]
            st, sp = tg == 0, tg == 7
            need("v", pr, t0)
            need("v", pr, t1)
            assert ("v", pr, t0) in done and ("v", pr, t1) in done, \
                ("V missing", pr, t0, t1)
            V0, V1 = VT[pr][t0 // 8], VT[pr][t1 // 8]
            nc.tensor.matmul(o0[:], V0[:, t0 % 8, 0, :], P0[:, 0:512],
                             start=st, stop=False)
            nc.tensor.matmul(o0[:], V1[:, t1 % 8, 0, :], P0[:, 512:1024],
                             start=False, stop=sp)
            nc.tensor.matmul(o1[:], V0[:, t0 % 8, 1, :], P1[:, 0:512],
                             start=st, stop=False)
            nc.tensor.matmul(o1[:], V1[:, t1 % 8, 1, :], P1[:, 512:1024],
                             start=False, stop=sp)
            if sp:
                emit_norm(pr, qb, o0, o1)
                del oaug_map[(pr, qb)]

        SLOTS = [(pr, qb, tg) for pr in range(4) for qb in range(4)
                 for tg in range(8)]
        for g, (pr, qb, tg) in enumerate(SLOTS):

            q0 = qb * 512
            t0, t1 = 2 * tg, 2 * tg + 1
            stage0 = ps.tile([128, 1024], F32, tag="stage", bufs=2,
                             name="stage0")
            stage1 = ps.tile([128, 1024], F32, tag="stage", bufs=2,
                             name="stage1")
            # scores S^T [keys, queries]; heads (2pr, 2pr+1) row-packed
            for _nb in range(8):
                need("k", pr, _nb)
                assert ("k", pr, _nb) in done, ("K missing", pr, _nb, g)
            for _nb in range(4 * (qb // 2), 4 * (qb // 2) + 4):
                need("q", pr, _nb)
                assert ("q", pr, _nb) in done, ("Q missing", pr, _nb, g)
            QTh = QT[pr][qb // 2]
            qh0 = (qb % 2) * 512
            nc.tensor.matmul(stage0[:, 0:512],
                             KT[pr][0:64, t0 * 128:(t0 + 1) * 128],
                             QTh[0:64, qh0:qh0 + 512],
                             start=True, stop=True, tile_position=(0, 0))
            nc.tensor.matmul(stage1[:, 0:512],
                             KT[pr][64:128, t0 * 128:(t0 + 1) * 128],
                             QTh[64:128, qh0:qh0 + 512],
                             start=True, stop=True, tile_position=(64, 0))
            nc.tensor.matmul(stage0[:, 512:1024],
                             KT[pr][0:64, t1 * 128:(t1 + 1) * 128],
                             QTh[0:64, qh0:qh0 + 512],
                             start=True, stop=True, tile_position=(0, 0))
            nc.tensor.matmul(stage1[:, 512:1024],
                             KT[pr][64:128, t1 * 128:(t1 + 1) * 128],
                             QTh[64:128, qh0:qh0 + 512],
                             start=True, stop=True, tile_position=(64, 0))
            if len(stagedq) == 2:
                emit_pv(stagedq.popleft())
            P0 = sb.tile([128, 1024], BF16, tag="p", bufs=6, name="P0")
            P1 = sb.tile([128, 1024], BF16, tag="p", bufs=6, name="P1")
            nc.scalar.activation(P0[:], stage0[:], EXP, bias=0.0, scale=SCALE)
            nc.scalar.activation(P1[:], stage1[:], EXP, bias=0.0, scale=SCALE)
            if dbg and g == 0:
                nc.sync.dma_start(dbgb[4:5, 0:1024], P0[0:1, :])
                nc.sync.dma_start(dbgb[5:6, 0:1024], P1[0:1, :])
            stagedq.append((pr, qb, tg, P0, P1, t0, t1))
            if pr == 3 and tg == 6 and qb >= 1:
                fills.extend(proj_groups(qb - 1))
            pace()
            for _ in range(2 if pr == 3 else 1):
                if pending:
                    pending.popleft()()

        while stagedq:
            emit_pv(stagedq.popleft())
        while pending:
            pending.popleft()()
        while fills:
            fills.popleft()[2]()
        for _, _, g2 in proj_groups(3, tail=True):
            g2()
    return nc


def _prepare_in_maps(x, qkv_w, qkv_b, proj_w):
    x = np.asarray(x, dtype=np.float32)
    wb = np.asarray(qkv_w, dtype=np.float32).astype(bfloat16)
    pwb = np.asarray(proj_w, dtype=np.float32).astype(bfloat16)
    qkv_b = np.asarray(qkv_b, dtype=np.float32)
    in_maps = []
    for c in range(8):
        b, g = c % 4, c // 4
        w0 = 512 * g
        in_maps.append({
            "xT": np.ascontiguousarray(x[b].T).astype(bfloat16),
            "wcat": np.ascontiguousarray(np.concatenate(
                [wb[:, w0:w0 + 512],
                 wb[:, 1024 + w0:1024 + w0 + 512],
                 wb[:, 2048 + w0:2048 + w0 + 512]], axis=1)),
            "qb": np.ascontiguousarray(qkv_b[w0:w0 + 512].reshape(4, 128).T),
            "kb": np.ascontiguousarray(
                qkv_b[1024 + w0:1024 + w0 + 512].reshape(4, 128).T),
            "pw": np.ascontiguousarray(pwb[w0:w0 + 512, :]),
        })
    return in_maps


def _gather(parts, qkv_b, proj_w, proj_b):
    const_row = (np.asarray(qkv_b)[2048:].astype(np.float64)
                 @ np.asarray(proj_w).astype(np.float64)
                 + np.asarray(proj_b).astype(np.float64))
    out = np.empty((B, N, C), np.float32)
    for b in range(B):
        out[b] = (np.asarray(parts[b]).astype(np.float64)
                  + np.asarray(parts[b + 4]).astype(np.float64)
                  + const_row).astype(np.float32)
    return out


def kernel(**inputs: np.ndarray) -> np.ndarray:
    x = np.asarray(inputs["x"], dtype=np.float32)
    qkv_w = np.asarray(inputs["qkv_w"], dtype=np.float32)
    qkv_b = np.asarray(inputs["qkv_b"], dtype=np.float32)
    proj_w = np.asarray(inputs["proj_w"], dtype=np.float32)
    proj_b = np.asarray(inputs["proj_b"], dtype=np.float32)

    in_maps = _prepare_in_maps(x, qkv_w, qkv_b, proj_w)
    nc = _build()
    nc.finalize()
    res = run_bass_kernel_spmd(nc, in_maps, list(range(8)))
    parts = [res.results[c]["out"] for c in range(8)]
    return _gather(parts, qkv_b, proj_w, proj_b)


if __name__ == "__main__":
    import tempfile
    import time

    from concourse.bass_utils import compile_bass_kernel

    t0 = time.time()
    nc = _build()
    nc.compile()
    with tempfile.TemporaryDirectory() as td:
        compile_bass_kernel(nc, td, neff_name="k.neff")
    print(f"COMPILE OK ({time.time() - t0:.0f}s)", flush=True)
